# revision 1
# baseline (speedup 1.0000x reference)
"""Trainium2 Bass kernel for nn_EntanglementPropagator (gnn_message_passing).

Math: the reference computes, for edges e=(src[e], dst[e]):
    eff_w[e,f]   = W[s,d,f] * cos(phase[s,d])
    signal[b,e,f]= x[b,s,f] * eff_w[e,f]
    out[b,n,f]   = (sum_{e: dst[e]==n} signal[b,e,f]) / max(out_deg[n],1)

Folding edge multiplicity M[s,d] (= # of (s,d) edges) and the 1/norm[d]
factor into a single per-(s,d) scale C[s,d] = cos(phase[s,d])*M[s,d]/norm[d]:

    out[b,d,f] = sum_s (W[s,d,f] * C[s,d]) * x[b,s,f]

i.e. F independent [B,N]x[N,DN] matmuls (contraction over source node s).

Sharding: dst-dimension split across the 8 cores (core c owns d in
[c*32,(c+1)*32)).  Each core reads W/8 + all of x (~17 MB) and writes out/8
(1 MB); no collectives needed.  The host only does layout work (slice /
transpose) plus preprocessing of the *integer* edge tensors (multiplicity /
degree counts); cos() and all heavy FP math run on device.

Key HW findings baked into the design:
  * fp32 matmuls self-load weights (no LDWEIGHTS pull-ahead) at 4 cycles
    per column, so many small matmuls are issue-bound (~168ns for 32x32).
    Packing 4 f-planes per matmul (M=N=128, ignoring the off-diagonal
    f-cross blocks) measures 318ns/matmul -> 2.1x less PE time total.
  * A packed operand must merge to a SINGLE free dim (walrus restriction),
    hence both W and X are kept f-major on SBUF ([s, f, d] / [s, f, b]),
    which also makes every DMA piece fully contiguous per partition.
  * PSUM accumulation groups must be contiguous on PE, so the two
    source-halves (kb) accumulate via SBUF: kb0 drains with a copy (ACT),
    kb1 with an add (DVE).
  * A matmul output must not cross a PSUM bank boundary.
"""

import numpy as np

import concourse.mybir as mybir
import concourse.tile as tile
from concourse import bacc
from concourse.bass_utils import run_bass_kernel_spmd

N = 256          # nodes
F = 256          # feature dim
B = 32           # batch
N_CORES = 8
DN = N // N_CORES        # dst nodes per core = 32
KB = 2                   # source-node partition blocks (s: 2 x 128)
FC = 32                  # f-range per PSUM chunk ([128, 8, 128] = 2 banks)
FP = 4                   # f-planes packed per matmul (M = FP*DN, N = FP*B)
F32 = mybir.dt.float32

HALF_PI = float(np.pi / 2.0)


def build_body(tc, w, xs, phm, out):
    """Emit one iteration of the kernel body.

    w   [N, F, DN]  DRAM  - W[:, d0:d0+DN, :] transposed to f-major
    xs  [N, F, B]   DRAM  - node_features transposed to [node, feat, batch]
    phm [2, N, DN]  DRAM  - phase[:, dsl] and M/norm scale (from int tensors)
    out [B, DN, F]  DRAM  - this core's output slice
    """
    nc = tc.nc

    with (
        tc.tile_pool(name="cpool", bufs=2) as cpool,
        tc.tile_pool(name="wpool", bufs=4) as wpool,
        tc.tile_pool(name="xpool", bufs=4) as xpool,
        tc.tile_pool(name="opool", bufs=1) as opool,
        tc.tile_pool(name="ppool", bufs=4, space="PSUM") as ppool,
    ):
        # --- per-(s,d) scale C = cos(phase) * M/norm, layout [s_part, d].
        # The Sin LUT is only accurate on ~[-pi, pi], so use the half-angle
        # form cos(x) = 2*sin^2(x/2 - pi/2) - 1 (argument stays in
        # [-pi/2, pi/2] for x in [0, 2pi]).
        bias_t = cpool.tile([128, 1], F32, tag="bias")
        nc.vector.memset(bias_t, -HALF_PI)
        phm_t = cpool.tile([128, 2, KB, DN], F32, tag="phm")
        nc.sync.dma_start(
            out=phm_t, in_=phm.rearrange("t (k p) d -> p t k d", k=KB))
        c_t = {}
        for kb in range(KB):
            c = cpool.tile([128, DN], F32, tag="c")
            nc.scalar.activation(out=c, in_=phm_t[:, 0, kb, :],
                                 func=mybir.ActivationFunctionType.Sin,
                                 bias=bias_t, scale=0.5)
            nc.vector.tensor_mul(out=c, in0=c, in1=c)
            nc.vector.tensor_scalar(out=c, in0=c, scalar1=2.0, scalar2=-1.0,
                                    op0=mybir.AluOpType.mult,
                                    op1=mybir.AluOpType.add)
            nc.vector.tensor_mul(out=c, in0=c, in1=phm_t[:, 1, kb, :])
            c_t[kb] = c

        # out_sb layout [d, b, f]: the packed matmul puts (f-plane, d) on
        # PSUM partitions, so drains land d-major; the out DMA restores the
        # [b, d, f] HBM order (partition stride = d stride).
        out_sb = opool.tile([DN, B, F], F32)

        # --- stream pieces and compute.  A piece is (kb, f0, f1): one W DMA
        # + scale + one X DMA + packed matmuls + PSUM drains.  All pieces
        # are fully contiguous per partition (f-major layouts), so piece
        # granularity is free - the tail pieces are small so that little
        # work remains after the last input byte lands.
        out_groups = [
            # (f-range of the out DMA, pieces)
            (slice(0, 128), [(0, 0, 128), (1, 0, 128)]),
            (slice(128, 256), [(0, 128, 256), (1, 128, 224), (1, 224, 256)]),
        ]
        for osl_f, pieces in out_groups:
            for kb, f0, f1 in pieces:
                fsl = slice(f0, f1)
                fw = f1 - f0
                ssl = slice(kb * 128, (kb + 1) * 128)
                wt = wpool.tile([128, 128, DN], F32, tag="w")
                wt = wt[:, :fw, :]
                nc.sync.dma_start(out=wt, in_=w[ssl, fsl, :])
                # W' = W * C  (broadcast C over f) on DVE
                nc.vector.tensor_mul(
                    out=wt, in0=wt,
                    in1=c_t[kb][:, None, :].broadcast_to([128, fw, DN]))

                xt = xpool.tile([128, 128, B], F32, tag="x")
                xt = xt[:, :fw, :]
                nc.sync.dma_start(out=xt, in_=xs[ssl, fsl, :])

                for ci in range(fw // FC):
                    # psum [(fj,d) = 128, g, (fi,b) = 128]; each matmul
                    # writes 512B/partition contiguous (bank-contained).
                    ps = ppool.tile([FP * DN, FC // FP, FP * B], F32)
                    for g in range(FC // FP):
                        fg = ci * FC + g * FP
                        nc.tensor.matmul(
                            ps[:, g],
                            lhsT=wt[:, fg:fg + FP, :].rearrange(
                                "s f d -> s (f d)"),
                            rhs=xt[:, fg:fg + FP, :].rearrange(
                                "s f b -> s (f b)"),
                            start=True, stop=True)
                    # drain diagonal (fi == fj) blocks; f = base + g*FP + fi
                    base = f0 + ci * FC
                    for fi in range(FP):
                        src = ps[fi * DN:(fi + 1) * DN, :,
                                 fi * B:(fi + 1) * B]
                        dst = out_sb[:, :, base + fi:base + FC:FP] \
                            .rearrange("d b g -> d g b")
                        if kb == 0:
                            # PSUM -> SBUF drain on ACT (keeps DVE free
                            # for the W-scaling muls)
                            nc.scalar.copy(out=dst, in_=src)
                        else:
                            nc.vector.tensor_add(out=dst, in0=dst, in1=src)
            # drain this group's f-range of the output.  Issued on the ACT
            # HWDGE queue: its sem wait (adds done) must not stall the
            # input stream on the sync queue.  (Measured alternatives that
            # LOSE: splitting this DMA across both rings, per-quarter out
            # groups, X pieces on the ACT ring.)
            nc.scalar.dma_start(
                out=out[:, :, osl_f].rearrange("b d f -> d b f"),
                in_=out_sb[:, :, osl_f])


def build_program(n_repeat=1, loop_k=None):
    nc = bacc.Bacc("TRN2", target_bir_lowering=False, debug=False,
                   num_devices=N_CORES)
    w = nc.dram_tensor("w", [N, F, DN], F32, kind="ExternalInput").ap()
    xs = nc.dram_tensor("xs", [N, F, B], F32, kind="ExternalInput").ap()
    phm = nc.dram_tensor("phm", [2, N, DN], F32, kind="ExternalInput").ap()
    out = nc.dram_tensor("out", [B, DN, F], F32, kind="ExternalOutput").ap()

    with tile.TileContext(nc) as tc:
        if loop_k is not None:
            # HW loop around the body - for wall-clock timing with enough
            # iterations to swamp the host<->device dispatch noise.
            with tc.For_i(0, loop_k, 1):
                for _ in range(n_repeat):
                    build_body(tc, w, xs, phm, out)
        else:
            for _ in range(n_repeat):
                build_body(tc, w, xs, phm, out)
    nc.compile()
    return nc


def host_prep(phase, src, dst):
    """Per-(s,d) multiplicity / out-degree normalization from the integer
    edge tensors.  Returns ms [N, N] float32 with ms[s,d] = M[s,d]/norm[d]."""
    src = np.asarray(src).astype(np.int64)
    dst = np.asarray(dst).astype(np.int64)
    counts = np.bincount(src, minlength=N).astype(np.float64)
    norm = np.maximum(counts, 1.0)                      # per-node out-degree
    mult = np.bincount(src * N + dst, minlength=N * N).astype(np.float64)
    mult = mult.reshape(N, N)
    ms = (mult / norm[None, :]).astype(np.float32)
    return ms


_PROGRAM_CACHE = {}


def get_program(n_repeat=1, loop_k=None):
    key = (n_repeat, loop_k)
    if key not in _PROGRAM_CACHE:
        _PROGRAM_CACHE[key] = build_program(n_repeat, loop_k)
    return _PROGRAM_CACHE[key]


def make_in_maps(node_features, W, phase, src, dst):
    node_features = np.asarray(node_features, dtype=np.float32)
    W = np.asarray(W, dtype=np.float32)
    phase = np.asarray(phase, dtype=np.float32)
    ms = host_prep(phase, src, dst)
    # f-major layouts (see module docstring): pure transposes, no math.
    xT = np.ascontiguousarray(node_features.transpose(1, 2, 0))  # [N, F, B]
    in_maps = []
    for c in range(N_CORES):
        dsl = slice(c * DN, (c + 1) * DN)
        in_maps.append({
            "w": np.ascontiguousarray(W[:, dsl, :].transpose(0, 2, 1)),
            "xs": xT,
            "phm": np.ascontiguousarray(
                np.stack([phase[:, dsl], ms[:, dsl]], axis=0)),
        })
    return in_maps


def kernel(node_features, W, phase, src, dst):
    nc = get_program(1)
    in_maps = make_in_maps(node_features, W, phase, src, dst)
    res = run_bass_kernel_spmd(nc, in_maps, list(range(N_CORES)))
    return np.concatenate([res.results[c]["out"] for c in range(N_CORES)],
                          axis=1)



# revision 2
# speedup vs baseline: 1.7582x; 1.7582x over previous
"""Trainium2 Bass kernel for nn_EntanglementPropagator (gnn_message_passing).

Math: with C[s,d] = cos(phase[s,d]) * M[s,d] / norm[d]  (M = edge
multiplicity, norm = clamped out-degree), the reference reduces to

    out[b,d,f] = sum_s (W[s,d,f] * C[s,d]) * x[b,s,f]

i.e. F independent [B,N] x [N,N] matmuls (contraction over source node s).

Sharding: FEATURE-dim split across the 8 cores (core c owns f in
[32c, 32c+32)).  Unlike dst-sharding, every input byte is read exactly
once across the machine: per core W slice 8 MB + x slice 1 MB + phase/ms
0.5 MB + out 1 MB ~= 10.5 MB, which at ~358 GB/s per-core HBM bandwidth
sets a ~29 us roofline (vs 17.5 MB -> ~49 us for dst-sharding).

Per-core compute structure (per feature f): out[b,:] += x[s,b]^T @ Wc[s,:]
with s split in two 128-partition blocks accumulated in PSUM.
Key design points:
  * W and x are cast fp32 -> bf16 DURING the DMA (SWDGE/gpsimd cast-DMA),
    so the DVE only does the C-scale multiply at bf16 2x rate (~10 us)
    instead of fp32 1x (~17.5 us), and SBUF traffic halves.
  * b (=32) sits on PSUM partitions; 4 features are packed into the four
    32-wide PE column groups via tile_position=(0,32j), so PSUM drains see
    all 128 partitions ([128,256] ACT copies, ~2.7 us total) instead of
    32-partition tiles (4x slower).
  * bf16 matmul streams rhs=Wc at 1 cycle/column (vs 4 for fp32): PE ~2-3 us.
  * The two s-halves accumulate in PSUM (start/stop pair per feature),
    which requires the (kb0,kb1) pieces of a f-chunk to arrive adjacently
    on the gpsimd DMA queue.
  * Tail pieces of the W stream are smaller so little work remains after
    the last input byte lands; out drains to DRAM in two halves on the
    ACT HWDGE ring.

The host only does layout work (slice/transpose/stack) plus preprocessing
of the *integer* edge tensors (multiplicity/degree bincounts); cos() and
all heavy FP math run on device.
"""

import numpy as np

import concourse.mybir as mybir
import concourse.tile as tile
from concourse import bacc
from concourse.bass_utils import run_bass_kernel_spmd

N = 256          # nodes
F = 256          # feature dim
B = 32           # batch
N_CORES = 8
FC = F // N_CORES        # features per core = 32
KB = 2                   # source-node partition blocks (s: 2 x 128)
CG = 4                   # features packed per PSUM tile (PE col groups)
NG = FC // CG            # feature groups per core = 8
F32 = mybir.dt.float32
BF16 = mybir.dt.bfloat16

HALF_PI = float(np.pi / 2.0)

# W stream f-chunks (per kb).  8+8+8+4+4 = 32; the tail pieces are small
# so that little work remains after the last input byte lands.
FCHUNKS = [(0, 8), (8, 16), (16, 24), (24, 28), (28, 32)]


def build_body(tc, w, xs, phm, out):
    """Emit one iteration of the kernel body.

    w   [N, FC, N]   DRAM fp32 - W[:, :, fsl] transposed to [s, f, d]
    xs  [N, FC, B]   DRAM fp32 - node_features[:, :, fsl] as [s, f, b]
    phm [2, N, N]    DRAM fp32 - phase and M/norm scale (int-derived)
    out [CG, B, NG, N] DRAM fp32 - psum-partition-major output layout:
                     out[j, b, g, d] = result[b, d, f=4g+j]
    """
    nc = tc.nc

    with (
        tc.tile_pool(name="cpool", bufs=2) as cpool,
        tc.tile_pool(name="xpool", bufs=2) as xpool,
        tc.tile_pool(name="wpool", bufs=4) as wpool,
        tc.tile_pool(name="opool", bufs=1) as opool,
        tc.tile_pool(name="ppool", bufs=4, space="PSUM") as ppool,
    ):
        # --- per-(s,d) scale C = cos(phase) * M/norm, bf16, layout
        # [s_part, d] per s-half.  The Sin LUT is only accurate on
        # ~[-pi, pi], so use cos(x) = 2*sin^2(x/2 - pi/2) - 1.
        bias_t = cpool.tile([128, 1], F32, tag="bias")
        nc.vector.memset(bias_t, -HALF_PI)
        phm_t = cpool.tile([128, 2, KB, N], F32, tag="phm")
        nc.sync.dma_start(
            out=phm_t, in_=phm.rearrange("t (k p) d -> p t k d", k=KB))
        c_t = {}
        for kb in range(KB):
            c = cpool.tile([128, N], F32, tag="c")
            nc.scalar.activation(out=c, in_=phm_t[:, 0, kb, :],
                                 func=mybir.ActivationFunctionType.Sin,
                                 bias=bias_t, scale=0.5)
            nc.vector.tensor_mul(out=c, in0=c, in1=c)
            nc.vector.tensor_scalar(out=c, in0=c, scalar1=2.0, scalar2=-1.0,
                                    op0=mybir.AluOpType.mult,
                                    op1=mybir.AluOpType.add)
            c16 = cpool.tile([128, N], BF16, tag="c16")
            nc.vector.tensor_mul(out=c16, in0=c, in1=phm_t[:, 1, kb, :])
            c_t[kb] = c16

        # --- x: fp32 -> bf16 cast-DMA, one piece per s-half.
        xt = {}
        for kb in range(KB):
            t = xpool.tile([128, FC, B], BF16, tag=f"x{kb}")
            nc.gpsimd.dma_start(
                out=t, in_=xs[kb * 128:(kb + 1) * 128, :, :])
            xt[kb] = t

        # out_sb [p=(j,b), g, d]: drains land partition-major; the host
        # unshards (transpose) so the out DMA is fully contiguous.
        out_sb = opool.tile([128, NG, N], F32)
        out_ap = out.rearrange("j b g d -> (j b) g d")

        # --- stream W pieces and compute.
        for f0, f1 in FCHUNKS:
            fw = f1 - f0
            wt = {}
            for kb in range(KB):
                t = wpool.tile([128, 8, N], BF16, tag="w")
                t = t[:, :fw, :]
                ssl = slice(kb * 128, (kb + 1) * 128)
                nc.gpsimd.dma_start(out=t, in_=w[ssl, f0:f1, :])
                # Wc = W * C (broadcast C over f) on DVE at bf16 2x rate
                nc.vector.tensor_mul(
                    out=t, in0=t,
                    in1=c_t[kb][:, None, :].broadcast_to([128, fw, N]))
                wt[kb] = t

            for g in range(f0 // CG, f1 // CG):
                ps = ppool.tile([128, N], F32)
                for j in range(CG):
                    fl = g * CG + j       # local feature index
                    fp = fl - f0          # index within this piece
                    for kb in range(KB):
                        nc.tensor.matmul(
                            ps[32 * j:32 * (j + 1), :],
                            lhsT=xt[kb][:, fl, :],
                            rhs=wt[kb][:, fp, :],
                            start=(kb == 0), stop=(kb == 1),
                            tile_position=(0, 32 * j))
                # PSUM -> SBUF drain on ACT (keeps DVE free for W-scaling)
                nc.scalar.copy(out=out_sb[:, g, :], in_=ps)
            # drain finished halves of the output on the ACT HWDGE ring
            if f1 == 16:
                nc.scalar.dma_start(out=out_ap[:, 0:4, :],
                                    in_=out_sb[:, 0:4, :])
            elif f1 == 32:
                nc.scalar.dma_start(out=out_ap[:, 4:8, :],
                                    in_=out_sb[:, 4:8, :])


def build_program(n_repeat=1, loop_k=None):
    nc = bacc.Bacc("TRN2", target_bir_lowering=False, debug=False,
                   num_devices=N_CORES)
    w = nc.dram_tensor("w", [N, FC, N], F32, kind="ExternalInput").ap()
    xs = nc.dram_tensor("xs", [N, FC, B], F32, kind="ExternalInput").ap()
    phm = nc.dram_tensor("phm", [2, N, N], F32, kind="ExternalInput").ap()
    out = nc.dram_tensor("out", [CG, B, NG, N], F32,
                         kind="ExternalOutput").ap()

    with tile.TileContext(nc) as tc:
        if loop_k is not None:
            with tc.For_i(0, loop_k, 1):
                for _ in range(n_repeat):
                    build_body(tc, w, xs, phm, out)
        else:
            for _ in range(n_repeat):
                build_body(tc, w, xs, phm, out)
    nc.compile()
    return nc


def host_prep(phase, src, dst):
    """Per-(s,d) multiplicity / out-degree normalization from the integer
    edge tensors.  Returns ms [N, N] float32 with ms[s,d] = M[s,d]/norm[d]."""
    src = np.asarray(src).astype(np.int64)
    dst = np.asarray(dst).astype(np.int64)
    counts = np.bincount(src, minlength=N).astype(np.float64)
    norm = np.maximum(counts, 1.0)                      # per-node out-degree
    mult = np.bincount(src * N + dst, minlength=N * N).astype(np.float64)
    mult = mult.reshape(N, N)
    ms = (mult / norm[None, :]).astype(np.float32)
    return ms


_PROGRAM_CACHE = {}


def get_program(n_repeat=1, loop_k=None):
    key = (n_repeat, loop_k)
    if key not in _PROGRAM_CACHE:
        _PROGRAM_CACHE[key] = build_program(n_repeat, loop_k)
    return _PROGRAM_CACHE[key]


def make_in_maps(node_features, W, phase, src, dst):
    node_features = np.asarray(node_features, dtype=np.float32)
    W = np.asarray(W, dtype=np.float32)
    phase = np.asarray(phase, dtype=np.float32)
    ms = host_prep(phase, src, dst)
    phm = np.ascontiguousarray(np.stack([phase, ms], axis=0))
    in_maps = []
    for c in range(N_CORES):
        fsl = slice(c * FC, (c + 1) * FC)
        in_maps.append({
            # [s, d, f] -> [s, f, d]
            "w": np.ascontiguousarray(W[:, :, fsl].transpose(0, 2, 1)),
            # [b, s, f] -> [s, f, b]
            "xs": np.ascontiguousarray(
                node_features[:, :, fsl].transpose(1, 2, 0)),
            "phm": phm,
        })
    return in_maps


def unshard(res_out):
    """Per-core out [CG, B, NG, N] (j, b, g, d) -> [B, N, FC] with f=4g+j."""
    return np.ascontiguousarray(
        res_out.transpose(1, 3, 2, 0).reshape(B, N, FC))


def kernel(node_features, W, phase, src, dst):
    nc = get_program(1)
    in_maps = make_in_maps(node_features, W, phase, src, dst)
    res = run_bass_kernel_spmd(nc, in_maps, list(range(N_CORES)))
    return np.concatenate(
        [unshard(res.results[c]["out"]) for c in range(N_CORES)], axis=2)


# revision 5
# speedup vs baseline: 1.9537x; 1.1112x over previous
"""Trainium2 Bass kernel for nn_EntanglementPropagator (gnn_message_passing).

Math: with C[s,d] = cos(phase[s,d]) * M[s,d] / norm[d]  (M = edge
multiplicity, norm = clamped out-degree), the reference reduces to

    out[b,d,f] = sum_s (W[s,d,f] * C[s,d]) * x[b,s,f]

i.e. F independent [B,N] x [N,N] matmuls (contraction over source node s).

Sharding: FEATURE-dim split across the 8 cores (core c owns f in
[32c, 32c+32)).  Unlike dst-sharding, every input byte is read exactly
once across the machine: per core W slice 8 MB + x slice 1 MB + phase/ms
0.5 MB + out 1 MB ~= 10.5 MB, which at ~358 GB/s per-core HBM bandwidth
sets a ~29 us roofline (vs 17.5 MB -> ~49 us for dst-sharding).

Per-core compute structure (per feature f): out[b,:] += x[s,b]^T @ Wc[s,:]
with s split in two 128-partition blocks accumulated in PSUM.
Key design points:
  * W and x are cast fp32 -> bf16 DURING the DMA (SWDGE/gpsimd cast-DMA),
    so the DVE only does the C-scale multiply at bf16 2x rate (~10 us)
    instead of fp32 1x (~17.5 us), and SBUF traffic halves.
  * b (=32) sits on PSUM partitions; 4 features are packed into the four
    32-wide PE column groups via tile_position=(0,32j), so PSUM drains see
    all 128 partitions ([128,256] ACT copies, ~2.7 us total) instead of
    32-partition tiles (4x slower).
  * bf16 matmul streams rhs=Wc at 1 cycle/column (vs 4 for fp32): PE ~2-3 us.
  * The two s-halves accumulate in PSUM (start/stop pair per feature),
    which requires the (kb0,kb1) pieces of a f-chunk to arrive adjacently
    on the gpsimd DMA queue.
  * Tail pieces of the W stream are smaller so little work remains after
    the last input byte lands; out drains to DRAM in two halves on the
    ACT HWDGE ring.

The host only does layout work (slice/transpose/stack) plus preprocessing
of the *integer* edge tensors (multiplicity/degree bincounts); cos() and
all heavy FP math run on device.
"""

import numpy as np

import concourse.mybir as mybir
import concourse.tile as tile
from concourse import bacc
from concourse.bass_utils import run_bass_kernel_spmd

N = 256          # nodes
F = 256          # feature dim
B = 32           # batch
N_CORES = 8
FC = F // N_CORES        # features per core = 32
KB = 2                   # source-node partition blocks (s: 2 x 128)
CG = 4                   # features packed per PSUM tile (PE col groups)
NG = FC // CG            # feature groups per core = 8
F32 = mybir.dt.float32
BF16 = mybir.dt.bfloat16

HALF_PI = float(np.pi / 2.0)

# W stream f-chunks (per kb).  8+8+8+4+4 = 32; the tail pieces are small
# so that little work remains after the last input byte lands.
FCHUNKS = [(0, 8), (8, 16), (16, 24), (24, 28), (28, 32)]


def build_body(tc, w, xs, phm, out):
    """Emit one iteration of the kernel body.

    w   [N, FC, N]   DRAM fp32 - W[:, :, fsl] transposed to [s, f, d]
    xs  [N, FC, B]   DRAM fp32 - node_features[:, :, fsl] as [s, f, b]
    phm [2, N, N]    DRAM fp32 - phase and M/norm scale (int-derived)
    out [CG, B, NG, N] DRAM fp32 - psum-partition-major output layout:
                     out[j, b, g, d] = result[b, d, f=4g+j]
    """
    nc = tc.nc

    with (
        tc.tile_pool(name="cpool", bufs=2) as cpool,
        tc.tile_pool(name="xpool", bufs=2) as xpool,
        tc.tile_pool(name="wpool", bufs=8) as wpool,
        tc.tile_pool(name="opool", bufs=1) as opool,
        tc.tile_pool(name="ppool", bufs=4, space="PSUM") as ppool,
    ):
        # --- per-(s,d) scale C = cos(phase) * M/norm, bf16, layout
        # [s_part, d] per s-half.  The Sin LUT is only accurate on
        # ~[-pi, pi], so use cos(x) = 2*sin^2(x/2 - pi/2) - 1.
        bias_t = cpool.tile([128, 1], F32, tag="bias")
        nc.vector.memset(bias_t, -HALF_PI)
        phm_t = cpool.tile([128, 2, KB, N], F32, tag="phm")
        nc.sync.dma_start(
            out=phm_t, in_=phm.rearrange("t (k p) d -> p t k d", k=KB))
        c_t = {}
        for kb in range(KB):
            c = cpool.tile([128, N], F32, tag="c")
            nc.scalar.activation(out=c, in_=phm_t[:, 0, kb, :],
                                 func=mybir.ActivationFunctionType.Sin,
                                 bias=bias_t, scale=0.5)
            nc.vector.tensor_mul(out=c, in0=c, in1=c)
            nc.vector.tensor_scalar(out=c, in0=c, scalar1=2.0, scalar2=-1.0,
                                    op0=mybir.AluOpType.mult,
                                    op1=mybir.AluOpType.add)
            c16 = cpool.tile([128, N], BF16, tag="c16")
            nc.vector.tensor_mul(out=c16, in0=c, in1=phm_t[:, 1, kb, :])
            c_t[kb] = c16

        # --- x: fp32 -> bf16 cast-DMA, one piece per s-half.
        xt = {}
        for kb in range(KB):
            t = xpool.tile([128, FC, B], BF16, tag=f"x{kb}")
            nc.gpsimd.dma_start(
                out=t, in_=xs[kb * 128:(kb + 1) * 128, :, :])
            xt[kb] = t

        # out_sb [p=(j,b), g, d]: drains land partition-major; the host
        # unshards (transpose) so the out DMA is fully contiguous.
        out_sb = opool.tile([128, NG, N], F32)
        out_ap = out.rearrange("j b g d -> (j b) g d")

        # --- stream W pieces and compute.
        for f0, f1 in FCHUNKS:
            fw = f1 - f0
            wt = {}
            for kb in range(KB):
                t = wpool.tile([128, 8, N], BF16, tag="w")
                t = t[:, :fw, :]
                ssl = slice(kb * 128, (kb + 1) * 128)
                nc.gpsimd.dma_start(out=t, in_=w[ssl, f0:f1, :])
                # Wc = W * C (broadcast C over f) on DVE at bf16 2x rate
                nc.vector.tensor_mul(
                    out=t, in0=t,
                    in1=c_t[kb][:, None, :].broadcast_to([128, fw, N]))
                wt[kb] = t

            for g in range(f0 // CG, f1 // CG):
                ps = ppool.tile([128, N], F32)
                for j in range(CG):
                    fl = g * CG + j       # local feature index
                    fp = fl - f0          # index within this piece
                    for kb in range(KB):
                        nc.tensor.matmul(
                            ps[32 * j:32 * (j + 1), :],
                            lhsT=xt[kb][:, fl, :],
                            rhs=wt[kb][:, fp, :],
                            start=(kb == 0), stop=(kb == 1),
                            tile_position=(0, 32 * j))
                # PSUM -> SBUF drain on ACT (keeps DVE free for W-scaling)
                nc.scalar.copy(out=out_sb[:, g, :], in_=ps)
                # drain finished pairs of groups on the ACT HWDGE ring;
                # small pieces keep the post-last-input tail short
                if g % 2 == 1:
                    nc.scalar.dma_start(out=out_ap[:, g - 1:g + 1, :],
                                        in_=out_sb[:, g - 1:g + 1, :])


def build_program(n_repeat=1, loop_k=None):
    nc = bacc.Bacc("TRN2", target_bir_lowering=False, debug=False,
                   num_devices=N_CORES)
    w = nc.dram_tensor("w", [N, FC, N], F32, kind="ExternalInput").ap()
    xs = nc.dram_tensor("xs", [N, FC, B], F32, kind="ExternalInput").ap()
    phm = nc.dram_tensor("phm", [2, N, N], F32, kind="ExternalInput").ap()
    out = nc.dram_tensor("out", [CG, B, NG, N], F32,
                         kind="ExternalOutput").ap()

    with tile.TileContext(nc) as tc:
        # Warmup Sin activation outside the loop so the one-time ACT
        # table load (~1.3us) is not paid inside every iteration.
        with tc.tile_pool(name="warm", bufs=1) as warm:
            wt = warm.tile([128, 1], F32)
            nc.vector.memset(wt, 0.0)
            nc.scalar.activation(out=wt, in_=wt,
                                 func=mybir.ActivationFunctionType.Sin)
        if loop_k is not None:
            with tc.For_i(0, loop_k, 1):
                for _ in range(n_repeat):
                    build_body(tc, w, xs, phm, out)
        else:
            for _ in range(n_repeat):
                build_body(tc, w, xs, phm, out)
    nc.compile()
    return nc


def host_prep(phase, src, dst):
    """Per-(s,d) multiplicity / out-degree normalization from the integer
    edge tensors.  Returns ms [N, N] float32 with ms[s,d] = M[s,d]/norm[d]."""
    src = np.asarray(src).astype(np.int64)
    dst = np.asarray(dst).astype(np.int64)
    counts = np.bincount(src, minlength=N).astype(np.float64)
    norm = np.maximum(counts, 1.0)                      # per-node out-degree
    mult = np.bincount(src * N + dst, minlength=N * N).astype(np.float64)
    mult = mult.reshape(N, N)
    ms = (mult / norm[None, :]).astype(np.float32)
    return ms


_PROGRAM_CACHE = {}


def get_program(n_repeat=1, loop_k=None):
    key = (n_repeat, loop_k)
    if key not in _PROGRAM_CACHE:
        _PROGRAM_CACHE[key] = build_program(n_repeat, loop_k)
    return _PROGRAM_CACHE[key]


def make_in_maps(node_features, W, phase, src, dst):
    node_features = np.asarray(node_features, dtype=np.float32)
    W = np.asarray(W, dtype=np.float32)
    phase = np.asarray(phase, dtype=np.float32)
    ms = host_prep(phase, src, dst)
    phm = np.ascontiguousarray(np.stack([phase, ms], axis=0))
    in_maps = []
    for c in range(N_CORES):
        fsl = slice(c * FC, (c + 1) * FC)
        in_maps.append({
            # [s, d, f] -> [s, f, d]
            "w": np.ascontiguousarray(W[:, :, fsl].transpose(0, 2, 1)),
            # [b, s, f] -> [s, f, b]
            "xs": np.ascontiguousarray(
                node_features[:, :, fsl].transpose(1, 2, 0)),
            "phm": phm,
        })
    return in_maps


def unshard(res_out):
    """Per-core out [CG, B, NG, N] (j, b, g, d) -> [B, N, FC] with f=4g+j."""
    return np.ascontiguousarray(
        res_out.transpose(1, 3, 2, 0).reshape(B, N, FC))


def kernel(node_features, W, phase, src, dst):
    nc = get_program(1)
    in_maps = make_in_maps(node_features, W, phase, src, dst)
    res = run_bass_kernel_spmd(nc, in_maps, list(range(N_CORES)))
    return np.concatenate(
        [unshard(res.results[c]["out"]) for c in range(N_CORES)], axis=2)


# revision 18
# speedup vs baseline: 2.0786x; 1.0639x over previous
"""Trainium2 Bass kernel for nn_EntanglementPropagator (gnn_message_passing).

Math: with C[s,d] = cos(phase[s,d]) * M[s,d] / norm[d]  (M = edge
multiplicity, norm = clamped out-degree), the reference reduces to

    out[b,d,f] = sum_s (W[s,d,f] * C[s,d]) * x[b,s,f]

i.e. F independent [B,N] x [N,N] matmuls (contraction over source node s).

Sharding: FEATURE-dim split across the 8 cores (core c owns f in
[32c, 32c+32)).  Unlike dst-sharding, every input byte is read exactly
once across the machine: per core W slice 8 MB + x slice 1 MB + phase/ms
0.5 MB + out 1 MB ~= 10.5 MB, which at ~358 GB/s per-core HBM bandwidth
sets a ~29 us roofline (vs 17.5 MB -> ~49 us for dst-sharding).

Per-core compute structure (per feature f): out[b,:] += x[s,b]^T @ Wc[s,:]
with s split in two 128-partition blocks accumulated in PSUM.
Key design points:
  * W and x are cast fp32 -> bf16 DURING the DMA (SWDGE/gpsimd cast-DMA),
    so the DVE only does the C-scale multiply at bf16 2x rate (~10 us)
    instead of fp32 1x (~17.5 us), and SBUF traffic halves.
  * b (=32) sits on PSUM partitions; 4 features are packed into the four
    32-wide PE column groups via tile_position=(0,32j), so PSUM drains see
    all 128 partitions ([128,256] ACT copies, ~2.7 us total) instead of
    32-partition tiles (4x slower).
  * bf16 matmul streams rhs=Wc at 1 cycle/column (vs 4 for fp32): PE ~2-3 us.
  * The two s-halves accumulate in PSUM (start/stop pair per feature),
    which requires the (kb0,kb1) pieces of a f-chunk to arrive adjacently
    on the gpsimd DMA queue.
  * Tail pieces of the W stream are smaller so little work remains after
    the last input byte lands; out drains to DRAM in two halves on the
    ACT HWDGE ring.

The host only does layout work (slice/transpose/stack) plus preprocessing
of the *integer* edge tensors (multiplicity/degree bincounts); cos() and
all heavy FP math run on device.
"""

import numpy as np

import concourse.mybir as mybir
import concourse.tile as tile
from concourse import bacc
from concourse.bass_utils import run_bass_kernel_spmd

N = 256          # nodes
F = 256          # feature dim
B = 32           # batch
N_CORES = 8
FC = F // N_CORES        # features per core = 32
KB = 2                   # source-node partition blocks (s: 2 x 128)
CG = 4                   # features packed per PSUM tile (PE col groups)
NG = FC // CG            # feature groups per core = 8
F32 = mybir.dt.float32
BF16 = mybir.dt.bfloat16

import os
K_DMA_ONLY = os.environ.get("K_DMA_ONLY", "0") == "1"   # skip compute (A/B)
K_WMODE = os.environ.get("K_WMODE", "cast")             # cast | plain
K_EMPTY = os.environ.get("K_EMPTY", "0") == "1"         # empty loop body
K_BIG = os.environ.get("K_BIG", "0") == "1"             # one 4MB W piece/kb
K_PHM16 = os.environ.get("K_PHM16", "1") == "1"         # phase/ms as fp16
F16 = mybir.dt.float16

HALF_PI = float(np.pi / 2.0)

# W stream f-chunks (per kb).  8+8+8+4+2+2 = 32; the tail pieces are small
# so that little work remains after the last input byte lands.
FCHUNKS = [(0, 8), (8, 16), (16, 24), (24, 28), (28, 30), (30, 32)]
# out-DMA pieces (group ranges), issued as soon as their groups are drained;
# the final pieces are single 128KB groups to shrink the tail.
OUT_PIECES = [(0, 2), (2, 4), (4, 6), (6, 7), (7, 8)]


def build_body(tc, w, xs, phm, out):
    """Emit one iteration of the kernel body.

    w   [N, FC, N]   DRAM fp32 - W[:, :, fsl] transposed to [s, f, d]
    xs  [N, FC, B]   DRAM fp32 - node_features[:, :, fsl] as [s, f, b]
    phm [2, N, N]    DRAM fp32 - phase and M/norm scale (int-derived)
    out [CG, B, NG, N] DRAM fp32 - psum-partition-major output layout:
                     out[j, b, g, d] = result[b, d, f=4g+j]
    """
    nc = tc.nc

    if K_EMPTY:
        with tc.tile_pool(name="epool", bufs=2) as epool:
            et = epool.tile([128, 1], F32)
            nc.vector.memset(et, 0.0)
        return

    with (
        tc.tile_pool(name="cpool", bufs=2) as cpool,
        tc.tile_pool(name="xpool", bufs=2) as xpool,
        tc.tile_pool(name="wpool", bufs=2 if K_BIG else 8) as wpool,
        tc.tile_pool(name="opool", bufs=1) as opool,
        tc.tile_pool(name="ppool", bufs=4, space="PSUM") as ppool,
    ):
        # --- per-(s,d) scale C = cos(phase) * M/norm, bf16, layout
        # [s_part, d] per s-half.  The Sin LUT is only accurate on
        # ~[-pi, pi], so use cos(x) = 2*sin^2(x/2 - pi/2) - 1.
        phdt = F16 if K_PHM16 else F32
        bias_t = cpool.tile([128, 1], F32, tag="bias")
        nc.vector.memset(bias_t, -HALF_PI)
        phm_t = cpool.tile([128, 2, KB, N], phdt, tag="phm")
        nc.sync.dma_start(
            out=phm_t, in_=phm.rearrange("t (k p) d -> p t k d", k=KB))
        c_t = {}
        for kb in range(KB):
            c = cpool.tile([128, N], F32, tag="c")
            nc.scalar.activation(out=c, in_=phm_t[:, 0, kb, :],
                                 func=mybir.ActivationFunctionType.Sin,
                                 bias=bias_t, scale=0.5)
            nc.vector.tensor_mul(out=c, in0=c, in1=c)
            nc.vector.tensor_scalar(out=c, in0=c, scalar1=2.0, scalar2=-1.0,
                                    op0=mybir.AluOpType.mult,
                                    op1=mybir.AluOpType.add)
            c16 = cpool.tile([128, N], BF16, tag="c16")
            nc.vector.tensor_mul(out=c16, in0=c, in1=phm_t[:, 1, kb, :])
            c_t[kb] = c16

        # --- x: fp32 -> bf16 cast-DMA, one piece per s-half.
        xt = {}
        for kb in range(KB):
            t = xpool.tile([128, FC, B], BF16, tag=f"x{kb}")
            nc.gpsimd.dma_start(
                out=t, in_=xs[kb * 128:(kb + 1) * 128, :, :])
            xt[kb] = t

        # out_sb [p=(j,b), g, d]: drains land partition-major; the host
        # unshards (transpose) so the out DMA is fully contiguous.
        out_sb = opool.tile([128, NG, N], F32)
        out_ap = out.rearrange("j b g d -> (j b) g d")
        if K_DMA_ONLY:
            nc.vector.memset(out_sb, 0.0)

        # --- stream W pieces and compute.  A group g (4 features) owns one
        # PSUM tile; its j-th col-block completes as soon as the piece
        # holding feature 4g+j has been scaled.  Groups drain (possibly in
        # partition-halves, for groups split across pieces) as their mms
        # finish, and out pieces go to DRAM as their groups drain.
        ps_of = {}                 # g -> psum tile
        drained_to = {}            # g -> next j to drain
        out_iter = iter(OUT_PIECES)
        next_out = next(out_iter, None)
        for f0, f1 in ([(0, FC)] if K_BIG else FCHUNKS):
            fw = f1 - f0
            wt = {}
            for kb in range(KB):
                wdt = BF16 if K_WMODE == "cast" else F32
                t = wpool.tile([128, FC if K_BIG else 8, N], wdt, tag="w")
                t = t[:, :fw, :]
                ssl = slice(kb * 128, (kb + 1) * 128)
                if K_WMODE == "cast":
                    nc.gpsimd.dma_start(out=t, in_=w[ssl, f0:f1, :])
                else:
                    nc.sync.dma_start(out=t, in_=w[ssl, f0:f1, :])
                if not K_DMA_ONLY:
                    # Wc = W * C (broadcast C over f) on DVE (bf16 2x rate)
                    nc.vector.tensor_mul(
                        out=t, in0=t,
                        in1=c_t[kb][:, None, :].broadcast_to([128, fw, N]))
                wt[kb] = t

            if not K_DMA_ONLY:
                for fl in range(f0, f1):
                    g, j = divmod(fl, CG)
                    if g not in ps_of:
                        ps_of[g] = ppool.tile([128, N], F32, name="ps",
                                              tag="ps")
                        drained_to[g] = 0
                    ps = ps_of[g]
                    for kb in range(KB):
                        nc.tensor.matmul(
                            ps[32 * j:32 * (j + 1), :],
                            lhsT=xt[kb][:, fl, :],
                            rhs=wt[kb][:, fp_ := fl - f0, :],
                            start=(kb == 0), stop=(kb == 1),
                            tile_position=(0, 32 * j))
                # drain every group col-range whose mms are now complete
                # (PSUM -> SBUF on ACT; keeps DVE free for W-scaling)
                for g in sorted(ps_of):
                    j_done = min(f1 - g * CG, CG)
                    j0 = drained_to[g]
                    if j_done > j0:
                        nc.scalar.copy(
                            out=out_sb[32 * j0:32 * j_done, g, :],
                            in_=ps_of[g][32 * j0:32 * j_done, :])
                        drained_to[g] = j_done
                    if j_done == CG:
                        del ps_of[g]
            # out pieces whose groups are fully drained go to DRAM on the
            # ACT HWDGE ring; small final pieces shrink the tail
            while next_out is not None and next_out[1] * CG <= f1:
                g0, g1 = next_out
                nc.scalar.dma_start(out=out_ap[:, g0:g1, :],
                                    in_=out_sb[:, g0:g1, :])
                next_out = next(out_iter, None)


def build_program(n_repeat=1, loop_k=None):
    nc = bacc.Bacc("TRN2", target_bir_lowering=False, debug=False,
                   num_devices=N_CORES)
    w = nc.dram_tensor("w", [N, FC, N], F32, kind="ExternalInput").ap()
    xs = nc.dram_tensor("xs", [N, FC, B], F32, kind="ExternalInput").ap()
    phm = nc.dram_tensor("phm", [2, N, N], F16 if K_PHM16 else F32,
                         kind="ExternalInput").ap()
    out = nc.dram_tensor("out", [CG, B, NG, N], F32,
                         kind="ExternalOutput").ap()

    with tile.TileContext(nc) as tc:
        # Warmup Sin activation outside the loop so the one-time ACT
        # table load (~1.3us) is not paid inside every iteration.
        with tc.tile_pool(name="warm", bufs=1) as warm:
            wt = warm.tile([128, 1], F32)
            nc.vector.memset(wt, 0.0)
            nc.scalar.activation(out=wt, in_=wt,
                                 func=mybir.ActivationFunctionType.Sin)
        if loop_k is not None:
            with tc.For_i(0, loop_k, 1):
                for _ in range(n_repeat):
                    build_body(tc, w, xs, phm, out)
        else:
            for _ in range(n_repeat):
                build_body(tc, w, xs, phm, out)
    nc.compile()
    return nc


def host_prep(phase, src, dst):
    """Per-(s,d) multiplicity / out-degree normalization from the integer
    edge tensors.  Returns ms [N, N] float32 with ms[s,d] = M[s,d]/norm[d]."""
    src = np.asarray(src).astype(np.int64)
    dst = np.asarray(dst).astype(np.int64)
    counts = np.bincount(src, minlength=N).astype(np.float64)
    norm = np.maximum(counts, 1.0)                      # per-node out-degree
    mult = np.bincount(src * N + dst, minlength=N * N).astype(np.float64)
    mult = mult.reshape(N, N)
    ms = (mult / norm[None, :]).astype(np.float32)
    return ms


_PROGRAM_CACHE = {}


def get_program(n_repeat=1, loop_k=None):
    key = (n_repeat, loop_k)
    if key not in _PROGRAM_CACHE:
        _PROGRAM_CACHE[key] = build_program(n_repeat, loop_k)
    return _PROGRAM_CACHE[key]


def make_in_maps(node_features, W, phase, src, dst):
    node_features = np.asarray(node_features, dtype=np.float32)
    W = np.asarray(W, dtype=np.float32)
    phase = np.asarray(phase, dtype=np.float32)
    ms = host_prep(phase, src, dst)
    phm = np.ascontiguousarray(np.stack([phase, ms], axis=0))
    if K_PHM16:
        phm = phm.astype(np.float16)
    in_maps = []
    for c in range(N_CORES):
        fsl = slice(c * FC, (c + 1) * FC)
        in_maps.append({
            # [s, d, f] -> [s, f, d]
            "w": np.ascontiguousarray(W[:, :, fsl].transpose(0, 2, 1)),
            # [b, s, f] -> [s, f, b]
            "xs": np.ascontiguousarray(
                node_features[:, :, fsl].transpose(1, 2, 0)),
            "phm": phm,
        })
    return in_maps


def unshard(res_out):
    """Per-core out [CG, B, NG, N] (j, b, g, d) -> [B, N, FC] with f=4g+j."""
    return np.ascontiguousarray(
        res_out.transpose(1, 3, 2, 0).reshape(B, N, FC))


def kernel(node_features, W, phase, src, dst):
    nc = get_program(1)
    in_maps = make_in_maps(node_features, W, phase, src, dst)
    res = run_bass_kernel_spmd(nc, in_maps, list(range(N_CORES)))
    return np.concatenate(
        [unshard(res.results[c]["out"]) for c in range(N_CORES)], axis=2)


# revision 23
# speedup vs baseline: 2.0821x; 1.0017x over previous
"""Trainium2 Bass kernel for nn_EntanglementPropagator (gnn_message_passing).

Math: with C[s,d] = cos(phase[s,d]) * M[s,d] / norm[d]  (M = edge
multiplicity, norm = clamped out-degree), the reference reduces to

    out[b,d,f] = sum_s (W[s,d,f] * C[s,d]) * x[b,s,f]

i.e. F independent [B,N] x [N,N] matmuls (contraction over source node s).

Sharding: FEATURE-dim split across the 8 cores (core c owns f in
[32c, 32c+32)).  Unlike dst-sharding, every input byte is read exactly
once across the machine: per core W slice 8 MB + x slice 1 MB + phase/ms
0.5 MB + out 1 MB ~= 10.5 MB, which at ~358 GB/s per-core HBM bandwidth
sets a ~29 us roofline (vs 17.5 MB -> ~49 us for dst-sharding).

Per-core compute structure (per feature f): out[b,:] += x[s,b]^T @ Wc[s,:]
with s split in two 128-partition blocks accumulated in PSUM.
Key design points:
  * W and x are cast fp32 -> bf16 DURING the DMA (SWDGE/gpsimd cast-DMA),
    so the DVE only does the C-scale multiply at bf16 2x rate (~10 us)
    instead of fp32 1x (~17.5 us), and SBUF traffic halves.
  * b (=32) sits on PSUM partitions; 4 features are packed into the four
    32-wide PE column groups via tile_position=(0,32j), so PSUM drains see
    all 128 partitions ([128,256] ACT copies, ~2.7 us total) instead of
    32-partition tiles (4x slower).
  * bf16 matmul streams rhs=Wc at 1 cycle/column (vs 4 for fp32): PE ~2-3 us.
  * The two s-halves accumulate in PSUM (start/stop pair per feature),
    which requires the (kb0,kb1) pieces of a f-chunk to arrive adjacently
    on the gpsimd DMA queue.
  * Tail pieces of the W stream are smaller so little work remains after
    the last input byte lands; out drains to DRAM in two halves on the
    ACT HWDGE ring.

The host only does layout work (slice/transpose/stack) plus preprocessing
of the *integer* edge tensors (multiplicity/degree bincounts); cos() and
all heavy FP math run on device.
"""

import numpy as np

import concourse.mybir as mybir
import concourse.tile as tile
from concourse import bacc
from concourse.bass_utils import run_bass_kernel_spmd

N = 256          # nodes
F = 256          # feature dim
B = 32           # batch
N_CORES = 8
FC = F // N_CORES        # features per core = 32
KB = 2                   # source-node partition blocks (s: 2 x 128)
CG = 4                   # features packed per PSUM tile (PE col groups)
NG = FC // CG            # feature groups per core = 8
F32 = mybir.dt.float32
BF16 = mybir.dt.bfloat16

import os
K_DMA_ONLY = os.environ.get("K_DMA_ONLY", "0") == "1"   # skip compute (A/B)
K_WMODE = os.environ.get("K_WMODE", "cast")             # cast | plain
K_EMPTY = os.environ.get("K_EMPTY", "0") == "1"         # empty loop body
K_BIG = os.environ.get("K_BIG", "0") == "1"             # one 4MB W piece/kb
K_PHM16 = os.environ.get("K_PHM16", "1") == "1"         # phase/ms as fp16
F16 = mybir.dt.float16

HALF_PI = float(np.pi / 2.0)

# W stream f-chunks (per kb).  16+8+4+2+2 = 32; big head pieces amortize
# per-DMA overheads, small tail pieces leave little work after the last
# input byte lands.
FCHUNKS = [(0, 16), (16, 24), (24, 28), (28, 30), (30, 32)]
FMAX = 16                # largest chunk width (wpool tile size)
# out-DMA pieces (group ranges), issued as soon as their groups are drained;
# the final pieces are single 128KB groups to shrink the tail.
OUT_PIECES = [(0, 2), (2, 4), (4, 6), (6, 7), (7, 8)]


def build_body(tc, w, xs, phm, out, bias_t):
    """Emit one iteration of the kernel body.

    w   [N, FC, N]   DRAM fp32 - W[:, :, fsl] transposed to [s, f, d]
    xs  [N, FC, B]   DRAM fp32 - node_features[:, :, fsl] as [s, f, b]
    phm [2, N, N]    DRAM fp32 - phase and M/norm scale (int-derived)
    out [CG, B, NG, N] DRAM fp32 - psum-partition-major output layout:
                     out[j, b, g, d] = result[b, d, f=4g+j]
    """
    nc = tc.nc

    if K_EMPTY:
        with tc.tile_pool(name="epool", bufs=2) as epool:
            et = epool.tile([128, 1], F32)
            nc.vector.memset(et, 0.0)
        return

    with (
        tc.tile_pool(name="cpool", bufs=2) as cpool,
        tc.tile_pool(name="xpool", bufs=2) as xpool,
        tc.tile_pool(name="wpool", bufs=2 if K_BIG else 8) as wpool,
        tc.tile_pool(name="opool", bufs=1) as opool,
        tc.tile_pool(name="ppool", bufs=4, space="PSUM") as ppool,
    ):
        # --- per-(s,d) scale C = cos(phase) * M/norm, bf16, layout
        # [s_part, d] per s-half.  The Sin LUT is only accurate on
        # ~[-pi, pi], so use cos(x) = 2*sin^2(x/2 - pi/2) - 1.
        phdt = F16 if K_PHM16 else F32
        phm_t = cpool.tile([128, 2, KB, N], phdt, tag="phm")
        nc.sync.dma_start(
            out=phm_t, in_=phm.rearrange("t (k p) d -> p t k d", k=KB))
        c_t = {}
        for kb in range(KB):
            c = cpool.tile([128, N], F32, tag="c")
            nc.scalar.activation(out=c, in_=phm_t[:, 0, kb, :],
                                 func=mybir.ActivationFunctionType.Sin,
                                 bias=bias_t, scale=0.5)
            nc.vector.tensor_mul(out=c, in0=c, in1=c)
            nc.vector.tensor_scalar(out=c, in0=c, scalar1=2.0, scalar2=-1.0,
                                    op0=mybir.AluOpType.mult,
                                    op1=mybir.AluOpType.add)
            c16 = cpool.tile([128, N], BF16, tag="c16")
            nc.vector.tensor_mul(out=c16, in0=c, in1=phm_t[:, 1, kb, :])
            c_t[kb] = c16

        # --- x: fp32 -> bf16 cast-DMA, one piece per s-half.
        xt = {}
        for kb in range(KB):
            t = xpool.tile([128, FC, B], BF16, tag=f"x{kb}")
            nc.gpsimd.dma_start(
                out=t, in_=xs[kb * 128:(kb + 1) * 128, :, :])
            xt[kb] = t

        # out_sb [p=(j,b), g, d]: drains land partition-major; the host
        # unshards (transpose) so the out DMA is fully contiguous.
        out_sb = opool.tile([128, NG, N], F32)
        out_ap = out.rearrange("j b g d -> (j b) g d")
        if K_DMA_ONLY:
            nc.vector.memset(out_sb, 0.0)

        # --- stream W pieces and compute.  A group g (4 features) owns one
        # PSUM tile; its j-th col-block completes as soon as the piece
        # holding feature 4g+j has been scaled.  Groups drain (possibly in
        # partition-halves, for groups split across pieces) as their mms
        # finish, and out pieces go to DRAM as their groups drain.
        ps_of = {}                 # g -> psum tile
        drained_to = {}            # g -> next j to drain
        out_iter = iter(OUT_PIECES)
        next_out = next(out_iter, None)
        for f0, f1 in ([(0, FC)] if K_BIG else FCHUNKS):
            fw = f1 - f0
            wt = {}
            for kb in range(KB):
                wdt = BF16 if K_WMODE == "cast" else F32
                t = wpool.tile([128, FC if K_BIG else FMAX, N], wdt, tag="w")
                t = t[:, :fw, :]
                ssl = slice(kb * 128, (kb + 1) * 128)
                if K_WMODE == "cast":
                    nc.gpsimd.dma_start(out=t, in_=w[ssl, f0:f1, :])
                else:
                    nc.sync.dma_start(out=t, in_=w[ssl, f0:f1, :])
                if not K_DMA_ONLY:
                    # Wc = W * C (broadcast C over f) on DVE (bf16 2x rate)
                    nc.vector.tensor_mul(
                        out=t, in0=t,
                        in1=c_t[kb][:, None, :].broadcast_to([128, fw, N]))
                wt[kb] = t

            if not K_DMA_ONLY:
                for fl in range(f0, f1):
                    g, j = divmod(fl, CG)
                    if g not in ps_of:
                        ps_of[g] = ppool.tile([128, N], F32, name="ps",
                                              tag="ps")
                        drained_to[g] = 0
                    ps = ps_of[g]
                    for kb in range(KB):
                        nc.tensor.matmul(
                            ps[32 * j:32 * (j + 1), :],
                            lhsT=xt[kb][:, fl, :],
                            rhs=wt[kb][:, fp_ := fl - f0, :],
                            start=(kb == 0), stop=(kb == 1),
                            tile_position=(0, 32 * j))
                # drain every group col-range whose mms are now complete
                # (PSUM -> SBUF on ACT; keeps DVE free for W-scaling)
                for g in sorted(ps_of):
                    j_done = min(f1 - g * CG, CG)
                    j0 = drained_to[g]
                    if j_done > j0:
                        nc.scalar.copy(
                            out=out_sb[32 * j0:32 * j_done, g, :],
                            in_=ps_of[g][32 * j0:32 * j_done, :])
                        drained_to[g] = j_done
                    if j_done == CG:
                        del ps_of[g]
            # out pieces whose groups are fully drained go to DRAM on the
            # ACT HWDGE ring; small final pieces shrink the tail
            while next_out is not None and next_out[1] * CG <= f1:
                g0, g1 = next_out
                nc.scalar.dma_start(out=out_ap[:, g0:g1, :],
                                    in_=out_sb[:, g0:g1, :])
                next_out = next(out_iter, None)


def build_program(n_repeat=1, loop_k=None):
    nc = bacc.Bacc("TRN2", target_bir_lowering=False, debug=False,
                   num_devices=N_CORES)
    w = nc.dram_tensor("w", [N, FC, N], F32, kind="ExternalInput").ap()
    xs = nc.dram_tensor("xs", [N, FC, B], F32, kind="ExternalInput").ap()
    phm = nc.dram_tensor("phm", [2, N, N], F16 if K_PHM16 else F32,
                         kind="ExternalInput").ap()
    out = nc.dram_tensor("out", [CG, B, NG, N], F32,
                         kind="ExternalOutput").ap()

    with tile.TileContext(nc) as tc:
        # Warmup Sin activation outside the loop so the one-time ACT
        # table load (~1.3us) is not paid inside every iteration.  The
        # constant Sin bias (-pi/2) is also hoisted so its memset does not
        # occupy the Pool queue (which emits the SWDGE W stream) per
        # iteration.
        with tc.tile_pool(name="constp", bufs=1) as constp:
            bias_t = constp.tile([128, 1], F32)
            nc.vector.memset(bias_t, -HALF_PI)
            warm_t = constp.tile([128, 1], F32)
            nc.scalar.activation(out=warm_t, in_=bias_t,
                                 func=mybir.ActivationFunctionType.Sin)
            if loop_k is not None:
                with tc.For_i(0, loop_k, 1):
                    for _ in range(n_repeat):
                        build_body(tc, w, xs, phm, out, bias_t)
            else:
                for _ in range(n_repeat):
                    build_body(tc, w, xs, phm, out, bias_t)
    nc.compile()
    return nc


def host_prep(phase, src, dst):
    """Per-(s,d) multiplicity / out-degree normalization from the integer
    edge tensors.  Returns ms [N, N] float32 with ms[s,d] = M[s,d]/norm[d]."""
    src = np.asarray(src).astype(np.int64)
    dst = np.asarray(dst).astype(np.int64)
    counts = np.bincount(src, minlength=N).astype(np.float64)
    norm = np.maximum(counts, 1.0)                      # per-node out-degree
    mult = np.bincount(src * N + dst, minlength=N * N).astype(np.float64)
    mult = mult.reshape(N, N)
    ms = (mult / norm[None, :]).astype(np.float32)
    return ms


_PROGRAM_CACHE = {}


def get_program(n_repeat=1, loop_k=None):
    key = (n_repeat, loop_k)
    if key not in _PROGRAM_CACHE:
        _PROGRAM_CACHE[key] = build_program(n_repeat, loop_k)
    return _PROGRAM_CACHE[key]


def make_in_maps(node_features, W, phase, src, dst):
    node_features = np.asarray(node_features, dtype=np.float32)
    W = np.asarray(W, dtype=np.float32)
    phase = np.asarray(phase, dtype=np.float32)
    ms = host_prep(phase, src, dst)
    phm = np.ascontiguousarray(np.stack([phase, ms], axis=0))
    if K_PHM16:
        phm = phm.astype(np.float16)
    in_maps = []
    for c in range(N_CORES):
        fsl = slice(c * FC, (c + 1) * FC)
        in_maps.append({
            # [s, d, f] -> [s, f, d]
            "w": np.ascontiguousarray(W[:, :, fsl].transpose(0, 2, 1)),
            # [b, s, f] -> [s, f, b]
            "xs": np.ascontiguousarray(
                node_features[:, :, fsl].transpose(1, 2, 0)),
            "phm": phm,
        })
    return in_maps


def unshard(res_out):
    """Per-core out [CG, B, NG, N] (j, b, g, d) -> [B, N, FC] with f=4g+j."""
    return np.ascontiguousarray(
        res_out.transpose(1, 3, 2, 0).reshape(B, N, FC))


def kernel(node_features, W, phase, src, dst):
    nc = get_program(1)
    in_maps = make_in_maps(node_features, W, phase, src, dst)
    res = run_bass_kernel_spmd(nc, in_maps, list(range(N_CORES)))
    return np.concatenate(
        [unshard(res.results[c]["out"]) for c in range(N_CORES)], axis=2)


# revision 24
# speedup vs baseline: 2.1444x; 1.0299x over previous
"""Trainium2 Bass kernel for nn_EntanglementPropagator (gnn_message_passing).

Math: with C[s,d] = cos(phase[s,d]) * M[s,d] / norm[d]  (M = edge
multiplicity, norm = clamped out-degree), the reference reduces to

    out[b,d,f] = sum_s (W[s,d,f] * C[s,d]) * x[b,s,f]

i.e. F independent [B,N] x [N,N] matmuls (contraction over source node s).

Sharding: FEATURE-dim split across the 8 cores (core c owns f in
[32c, 32c+32)).  Unlike dst-sharding, every input byte is read exactly
once across the machine: per core W slice 8 MB + x slice 1 MB + phase/ms
0.5 MB + out 1 MB ~= 10.5 MB, which at ~358 GB/s per-core HBM bandwidth
sets a ~29 us roofline (vs 17.5 MB -> ~49 us for dst-sharding).

Per-core compute structure (per feature f): out[b,:] += x[s,b]^T @ Wc[s,:]
with s split in two 128-partition blocks accumulated in PSUM.
Key design points:
  * W and x are cast fp32 -> bf16 DURING the DMA (SWDGE/gpsimd cast-DMA),
    so the DVE only does the C-scale multiply at bf16 2x rate (~10 us)
    instead of fp32 1x (~17.5 us), and SBUF traffic halves.
  * b (=32) sits on PSUM partitions; 4 features are packed into the four
    32-wide PE column groups via tile_position=(0,32j), so PSUM drains see
    all 128 partitions ([128,256] ACT copies, ~2.7 us total) instead of
    32-partition tiles (4x slower).
  * bf16 matmul streams rhs=Wc at 1 cycle/column (vs 4 for fp32): PE ~2-3 us.
  * The two s-halves accumulate in PSUM (start/stop pair per feature),
    which requires the (kb0,kb1) pieces of a f-chunk to arrive adjacently
    on the gpsimd DMA queue.
  * Tail pieces of the W stream are smaller so little work remains after
    the last input byte lands; out drains to DRAM in two halves on the
    ACT HWDGE ring.

The host only does layout work (slice/transpose/stack) plus preprocessing
of the *integer* edge tensors (multiplicity/degree bincounts); cos() and
all heavy FP math run on device.
"""

import numpy as np

import concourse.mybir as mybir
import concourse.tile as tile
from concourse import bacc
from concourse.bass_utils import run_bass_kernel_spmd

N = 256          # nodes
F = 256          # feature dim
B = 32           # batch
N_CORES = 8
FC = F // N_CORES        # features per core = 32
KB = 2                   # source-node partition blocks (s: 2 x 128)
CG = 4                   # features packed per PSUM tile (PE col groups)
NG = FC // CG            # feature groups per core = 8
F32 = mybir.dt.float32
BF16 = mybir.dt.bfloat16

import os
K_DMA_ONLY = os.environ.get("K_DMA_ONLY", "0") == "1"   # skip compute (A/B)
K_WMODE = os.environ.get("K_WMODE", "cast")             # cast | plain
K_EMPTY = os.environ.get("K_EMPTY", "0") == "1"         # empty loop body
K_BIG = os.environ.get("K_BIG", "0") == "1"             # one 4MB W piece/kb
K_PHM16 = os.environ.get("K_PHM16", "1") == "1"         # phase/ms as fp16
K_OUT16 = os.environ.get("K_OUT16", "1") == "1"         # out as fp16 (host widens)
F16 = mybir.dt.float16

HALF_PI = float(np.pi / 2.0)

# W stream f-chunks (per kb).  16+8+4+2+2 = 32; big head pieces amortize
# per-DMA overheads, small tail pieces leave little work after the last
# input byte lands.
FCHUNKS = [(0, 16), (16, 24), (24, 28), (28, 30), (30, 32)]
FMAX = 16                # largest chunk width (wpool tile size)
# out-DMA pieces (group ranges), issued as soon as their groups are drained;
# the final pieces are single 128KB groups to shrink the tail.
OUT_PIECES = [(0, 2), (2, 4), (4, 6), (6, 7), (7, 8)]


def build_body(tc, w, xs, phm, out, bias_t):
    """Emit one iteration of the kernel body.

    w   [N, FC, N]   DRAM fp32 - W[:, :, fsl] transposed to [s, f, d]
    xs  [N, FC, B]   DRAM fp32 - node_features[:, :, fsl] as [s, f, b]
    phm [2, N, N]    DRAM fp32 - phase and M/norm scale (int-derived)
    out [CG, B, NG, N] DRAM fp32 - psum-partition-major output layout:
                     out[j, b, g, d] = result[b, d, f=4g+j]
    """
    nc = tc.nc

    if K_EMPTY:
        with tc.tile_pool(name="epool", bufs=2) as epool:
            et = epool.tile([128, 1], F32)
            nc.vector.memset(et, 0.0)
        return

    with (
        tc.tile_pool(name="cpool", bufs=2) as cpool,
        tc.tile_pool(name="xpool", bufs=2) as xpool,
        tc.tile_pool(name="wpool", bufs=2 if K_BIG else 8) as wpool,
        tc.tile_pool(name="opool", bufs=1) as opool,
        tc.tile_pool(name="ppool", bufs=4, space="PSUM") as ppool,
    ):
        # --- per-(s,d) scale C = cos(phase) * M/norm, bf16, layout
        # [s_part, d] per s-half.  The Sin LUT is only accurate on
        # ~[-pi, pi], so use cos(x) = 2*sin^2(x/2 - pi/2) - 1.
        phdt = F16 if K_PHM16 else F32
        phm_t = cpool.tile([128, 2, KB, N], phdt, tag="phm")
        nc.sync.dma_start(
            out=phm_t, in_=phm.rearrange("t (k p) d -> p t k d", k=KB))
        c_t = {}
        for kb in range(KB):
            c = cpool.tile([128, N], F32, tag="c")
            nc.scalar.activation(out=c, in_=phm_t[:, 0, kb, :],
                                 func=mybir.ActivationFunctionType.Sin,
                                 bias=bias_t, scale=0.5)
            nc.vector.tensor_mul(out=c, in0=c, in1=c)
            nc.vector.tensor_scalar(out=c, in0=c, scalar1=2.0, scalar2=-1.0,
                                    op0=mybir.AluOpType.mult,
                                    op1=mybir.AluOpType.add)
            c16 = cpool.tile([128, N], BF16, tag="c16")
            nc.vector.tensor_mul(out=c16, in0=c, in1=phm_t[:, 1, kb, :])
            c_t[kb] = c16

        # --- x: fp32 -> bf16 cast-DMA, one piece per s-half.
        xt = {}
        for kb in range(KB):
            t = xpool.tile([128, FC, B], BF16, tag=f"x{kb}")
            nc.gpsimd.dma_start(
                out=t, in_=xs[kb * 128:(kb + 1) * 128, :, :])
            xt[kb] = t

        # out_sb [p=(j,b), g, d]: drains land partition-major; the host
        # unshards (transpose) so the out DMA is fully contiguous.
        out_sb = opool.tile([128, NG, N], F16 if K_OUT16 else F32)
        out_ap = out.rearrange("j b g d -> (j b) g d")
        if K_DMA_ONLY:
            nc.vector.memset(out_sb, 0.0)

        # --- stream W pieces and compute.  A group g (4 features) owns one
        # PSUM tile; its j-th col-block completes as soon as the piece
        # holding feature 4g+j has been scaled.  Groups drain (possibly in
        # partition-halves, for groups split across pieces) as their mms
        # finish, and out pieces go to DRAM as their groups drain.
        ps_of = {}                 # g -> psum tile
        drained_to = {}            # g -> next j to drain
        out_iter = iter(OUT_PIECES)
        next_out = next(out_iter, None)
        for f0, f1 in ([(0, FC)] if K_BIG else FCHUNKS):
            fw = f1 - f0
            wt = {}
            for kb in range(KB):
                wdt = BF16 if K_WMODE == "cast" else F32
                t = wpool.tile([128, FC if K_BIG else FMAX, N], wdt, tag="w")
                t = t[:, :fw, :]
                ssl = slice(kb * 128, (kb + 1) * 128)
                if K_WMODE == "cast":
                    nc.gpsimd.dma_start(out=t, in_=w[ssl, f0:f1, :])
                else:
                    nc.sync.dma_start(out=t, in_=w[ssl, f0:f1, :])
                if not K_DMA_ONLY:
                    # Wc = W * C (broadcast C over f) on DVE (bf16 2x rate)
                    nc.vector.tensor_mul(
                        out=t, in0=t,
                        in1=c_t[kb][:, None, :].broadcast_to([128, fw, N]))
                wt[kb] = t

            if not K_DMA_ONLY:
                for fl in range(f0, f1):
                    g, j = divmod(fl, CG)
                    if g not in ps_of:
                        ps_of[g] = ppool.tile([128, N], F32, name="ps",
                                              tag="ps")
                        drained_to[g] = 0
                    ps = ps_of[g]
                    for kb in range(KB):
                        nc.tensor.matmul(
                            ps[32 * j:32 * (j + 1), :],
                            lhsT=xt[kb][:, fl, :],
                            rhs=wt[kb][:, fp_ := fl - f0, :],
                            start=(kb == 0), stop=(kb == 1),
                            tile_position=(0, 32 * j))
                # drain every group col-range whose mms are now complete
                # (PSUM -> SBUF on ACT; keeps DVE free for W-scaling)
                for g in sorted(ps_of):
                    j_done = min(f1 - g * CG, CG)
                    j0 = drained_to[g]
                    if j_done > j0:
                        nc.scalar.copy(
                            out=out_sb[32 * j0:32 * j_done, g, :],
                            in_=ps_of[g][32 * j0:32 * j_done, :])
                        drained_to[g] = j_done
                    if j_done == CG:
                        del ps_of[g]
            # out pieces whose groups are fully drained go to DRAM on the
            # ACT HWDGE ring; small final pieces shrink the tail
            while next_out is not None and next_out[1] * CG <= f1:
                g0, g1 = next_out
                nc.scalar.dma_start(out=out_ap[:, g0:g1, :],
                                    in_=out_sb[:, g0:g1, :])
                next_out = next(out_iter, None)


def build_program(n_repeat=1, loop_k=None):
    nc = bacc.Bacc("TRN2", target_bir_lowering=False, debug=False,
                   num_devices=N_CORES)
    w = nc.dram_tensor("w", [N, FC, N], F32, kind="ExternalInput").ap()
    xs = nc.dram_tensor("xs", [N, FC, B], F32, kind="ExternalInput").ap()
    phm = nc.dram_tensor("phm", [2, N, N], F16 if K_PHM16 else F32,
                         kind="ExternalInput").ap()
    out = nc.dram_tensor("out", [CG, B, NG, N], F16 if K_OUT16 else F32,
                         kind="ExternalOutput").ap()

    with tile.TileContext(nc) as tc:
        # Warmup Sin activation outside the loop so the one-time ACT
        # table load (~1.3us) is not paid inside every iteration.  The
        # constant Sin bias (-pi/2) is also hoisted so its memset does not
        # occupy the Pool queue (which emits the SWDGE W stream) per
        # iteration.
        with tc.tile_pool(name="constp", bufs=1) as constp:
            bias_t = constp.tile([128, 1], F32)
            nc.vector.memset(bias_t, -HALF_PI)
            warm_t = constp.tile([128, 1], F32)
            nc.scalar.activation(out=warm_t, in_=bias_t,
                                 func=mybir.ActivationFunctionType.Sin)
            if loop_k is not None:
                with tc.For_i(0, loop_k, 1):
                    for _ in range(n_repeat):
                        build_body(tc, w, xs, phm, out, bias_t)
            else:
                for _ in range(n_repeat):
                    build_body(tc, w, xs, phm, out, bias_t)
    nc.compile()
    return nc


def host_prep(phase, src, dst):
    """Per-(s,d) multiplicity / out-degree normalization from the integer
    edge tensors.  Returns ms [N, N] float32 with ms[s,d] = M[s,d]/norm[d]."""
    src = np.asarray(src).astype(np.int64)
    dst = np.asarray(dst).astype(np.int64)
    counts = np.bincount(src, minlength=N).astype(np.float64)
    norm = np.maximum(counts, 1.0)                      # per-node out-degree
    mult = np.bincount(src * N + dst, minlength=N * N).astype(np.float64)
    mult = mult.reshape(N, N)
    ms = (mult / norm[None, :]).astype(np.float32)
    return ms


_PROGRAM_CACHE = {}


def get_program(n_repeat=1, loop_k=None):
    key = (n_repeat, loop_k)
    if key not in _PROGRAM_CACHE:
        _PROGRAM_CACHE[key] = build_program(n_repeat, loop_k)
    return _PROGRAM_CACHE[key]


def make_in_maps(node_features, W, phase, src, dst):
    node_features = np.asarray(node_features, dtype=np.float32)
    W = np.asarray(W, dtype=np.float32)
    phase = np.asarray(phase, dtype=np.float32)
    ms = host_prep(phase, src, dst)
    phm = np.ascontiguousarray(np.stack([phase, ms], axis=0))
    if K_PHM16:
        phm = phm.astype(np.float16)
    in_maps = []
    for c in range(N_CORES):
        fsl = slice(c * FC, (c + 1) * FC)
        in_maps.append({
            # [s, d, f] -> [s, f, d]
            "w": np.ascontiguousarray(W[:, :, fsl].transpose(0, 2, 1)),
            # [b, s, f] -> [s, f, b]
            "xs": np.ascontiguousarray(
                node_features[:, :, fsl].transpose(1, 2, 0)),
            "phm": phm,
        })
    return in_maps


def unshard(res_out):
    """Per-core out [CG, B, NG, N] (j, b, g, d) -> [B, N, FC] with f=4g+j."""
    return np.ascontiguousarray(
        res_out.astype(np.float32).transpose(1, 3, 2, 0).reshape(B, N, FC))


def kernel(node_features, W, phase, src, dst):
    nc = get_program(1)
    in_maps = make_in_maps(node_features, W, phase, src, dst)
    res = run_bass_kernel_spmd(nc, in_maps, list(range(N_CORES)))
    return np.concatenate(
        [unshard(res.results[c]["out"]) for c in range(N_CORES)], axis=2)


# revision 27
# speedup vs baseline: 2.2070x; 1.0292x over previous
"""Trainium2 Bass kernel for nn_EntanglementPropagator (gnn_message_passing).

Math: with C[s,d] = cos(phase[s,d]) * M[s,d] / norm[d]  (M = edge
multiplicity, norm = clamped out-degree), the reference reduces to

    out[b,d,f] = sum_s (W[s,d,f] * C[s,d]) * x[b,s,f]

i.e. F independent [B,N] x [N,N] matmuls (contraction over source node s).

Sharding: FEATURE-dim split across the 8 cores (core c owns f in
[32c, 32c+32)).  Unlike dst-sharding, every input byte is read exactly
once across the machine: per core W slice 8 MB + x slice 1 MB + phase/ms
0.5 MB + out 1 MB ~= 10.5 MB, which at ~358 GB/s per-core HBM bandwidth
sets a ~29 us roofline (vs 17.5 MB -> ~49 us for dst-sharding).

Per-core compute structure (per feature f): out[b,:] += x[s,b]^T @ Wc[s,:]
with s split in two 128-partition blocks accumulated in PSUM.
Key design points:
  * W and x are cast fp32 -> bf16 DURING the DMA (SWDGE/gpsimd cast-DMA),
    so the DVE only does the C-scale multiply at bf16 2x rate (~10 us)
    instead of fp32 1x (~17.5 us), and SBUF traffic halves.
  * b (=32) sits on PSUM partitions; 4 features are packed into the four
    32-wide PE column groups via tile_position=(0,32j), so PSUM drains see
    all 128 partitions ([128,256] ACT copies, ~2.7 us total) instead of
    32-partition tiles (4x slower).
  * bf16 matmul streams rhs=Wc at 1 cycle/column (vs 4 for fp32): PE ~2-3 us.
  * The two s-halves accumulate in PSUM (start/stop pair per feature),
    which requires the (kb0,kb1) pieces of a f-chunk to arrive adjacently
    on the gpsimd DMA queue.
  * Tail pieces of the W stream are smaller so little work remains after
    the last input byte lands; out drains to DRAM in two halves on the
    ACT HWDGE ring.

The host only does layout work (slice/transpose/stack) plus preprocessing
of the *integer* edge tensors (multiplicity/degree bincounts); cos() and
all heavy FP math run on device.
"""

import numpy as np

import concourse.mybir as mybir
import concourse.tile as tile
from concourse import bacc
from concourse.bass_utils import run_bass_kernel_spmd

N = 256          # nodes
F = 256          # feature dim
B = 32           # batch
N_CORES = 8
FC = F // N_CORES        # features per core = 32
KB = 2                   # source-node partition blocks (s: 2 x 128)
CG = 4                   # features packed per PSUM tile (PE col groups)
NG = FC // CG            # feature groups per core = 8
F32 = mybir.dt.float32
BF16 = mybir.dt.bfloat16

import os
K_DMA_ONLY = os.environ.get("K_DMA_ONLY", "0") == "1"   # skip compute (A/B)
K_WMODE = os.environ.get("K_WMODE", "cast")             # cast | plain
K_EMPTY = os.environ.get("K_EMPTY", "0") == "1"         # empty loop body
K_BIG = os.environ.get("K_BIG", "0") == "1"             # one 4MB W piece/kb
K_PHM16 = os.environ.get("K_PHM16", "1") == "1"         # phase/ms as fp16
K_OUT16 = os.environ.get("K_OUT16", "1") == "1"         # out as fp16 (host widens)
F16 = mybir.dt.float16

HALF_PI = float(np.pi / 2.0)

# W stream f-chunks (per kb).  16+8+4+2+2 = 32; big head pieces amortize
# per-DMA overheads, small tail pieces leave little work after the last
# input byte lands.
FCHUNKS = [(0, 16), (16, 24), (24, 28), (28, 30), (30, 32)]
FMAX = 16                # largest chunk width (wpool tile size)
# out-DMA pieces (group ranges), issued as soon as their groups are drained;
# the final pieces are single 128KB groups to shrink the tail.
OUT_PIECES = [(0, 2), (2, 4), (4, 6), (6, 7), (7, 8)]


def build_body(tc, w, xs, phm, out, bias_t):
    """Emit one iteration of the kernel body.

    w   [N, FC, N]   DRAM fp32 - W[:, :, fsl] transposed to [s, f, d]
    xs  [N, FC, B]   DRAM fp32 - node_features[:, :, fsl] as [s, f, b]
    phm [2, N, N]    DRAM fp32 - phase and M/norm scale (int-derived)
    out [CG, B, NG, N] DRAM fp32 - psum-partition-major output layout:
                     out[j, b, g, d] = result[b, d, f=4g+j]
    """
    nc = tc.nc

    if K_EMPTY:
        with tc.tile_pool(name="epool", bufs=2) as epool:
            et = epool.tile([128, 1], F32)
            nc.vector.memset(et, 0.0)
        return

    with (
        tc.tile_pool(name="cpool", bufs=2) as cpool,
        tc.tile_pool(name="xpool", bufs=2) as xpool,
        tc.tile_pool(name="wpool", bufs=2 if K_BIG else 4) as wpool,
        tc.tile_pool(name="opool", bufs=1) as opool,
        tc.tile_pool(name="ppool", bufs=4, space="PSUM") as ppool,
    ):
        # --- per-(s,d) scale C = cos(phase) * M/norm, bf16, layout
        # [s_part, d] per s-half.  The Sin LUT is only accurate on
        # ~[-pi, pi], so use cos(x) = 2*sin^2(x/2 - pi/2) - 1.
        phdt = F16 if K_PHM16 else F32
        phm_t = cpool.tile([128, 2, KB, N], phdt, tag="phm")
        nc.sync.dma_start(
            out=phm_t, in_=phm.rearrange("t (k p) d -> p t k d", k=KB))
        c_t = {}
        for kb in range(KB):
            c = cpool.tile([128, N], F32, tag="c")
            nc.scalar.activation(out=c, in_=phm_t[:, 0, kb, :],
                                 func=mybir.ActivationFunctionType.Sin,
                                 bias=bias_t, scale=0.5)
            nc.vector.tensor_mul(out=c, in0=c, in1=c)
            nc.vector.tensor_scalar(out=c, in0=c, scalar1=2.0, scalar2=-1.0,
                                    op0=mybir.AluOpType.mult,
                                    op1=mybir.AluOpType.add)
            c16 = cpool.tile([128, N], BF16, tag="c16")
            nc.vector.tensor_mul(out=c16, in0=c, in1=phm_t[:, 1, kb, :])
            c_t[kb] = c16

        # --- x: fp32 -> bf16 cast-DMA, both s-halves in one transfer.
        x_t = xpool.tile([128, KB, FC, B], BF16, tag="x")
        nc.gpsimd.dma_start(
            out=x_t, in_=xs.rearrange("(k p) f b -> p k f b", k=KB))
        xt = {kb: x_t[:, kb] for kb in range(KB)}

        # out_sb [p=(j,b), g, d]: drains land partition-major; the host
        # unshards (transpose) so the out DMA is fully contiguous.
        out_sb = opool.tile([128, NG, N], F16 if K_OUT16 else F32)
        out_ap = out.rearrange("j b g d -> (j b) g d")
        if K_DMA_ONLY:
            nc.vector.memset(out_sb, 0.0)

        # --- stream W pieces and compute.  A group g (4 features) owns one
        # PSUM tile; its j-th col-block completes as soon as the piece
        # holding feature 4g+j has been scaled.  Groups drain (possibly in
        # partition-halves, for groups split across pieces) as their mms
        # finish, and out pieces go to DRAM as their groups drain.
        ps_of = {}                 # g -> psum tile
        drained_to = {}            # g -> next j to drain
        out_iter = iter(OUT_PIECES)
        next_out = next(out_iter, None)
        w_r = w.rearrange("(k p) f d -> p k f d", k=KB)
        for f0, f1 in ([(0, FC)] if K_BIG else FCHUNKS):
            fw = f1 - f0
            wdt = BF16 if K_WMODE == "cast" else F32
            t = wpool.tile([128, KB, FC if K_BIG else FMAX, N], wdt, tag="w")
            t = t[:, :, :fw, :]
            if K_WMODE == "cast":
                nc.gpsimd.dma_start(out=t, in_=w_r[:, :, f0:f1, :])
            else:
                nc.sync.dma_start(out=t, in_=w_r[:, :, f0:f1, :])
            wt = {}
            for kb in range(KB):
                wt[kb] = t[:, kb]
                if not K_DMA_ONLY:
                    # Wc = W * C (broadcast C over f) on DVE (bf16 2x rate)
                    nc.vector.tensor_mul(
                        out=wt[kb], in0=wt[kb],
                        in1=c_t[kb][:, None, :].broadcast_to([128, fw, N]))

            if not K_DMA_ONLY:
                for fl in range(f0, f1):
                    g, j = divmod(fl, CG)
                    if g not in ps_of:
                        ps_of[g] = ppool.tile([128, N], F32, name="ps",
                                              tag="ps")
                        drained_to[g] = 0
                    ps = ps_of[g]
                    for kb in range(KB):
                        nc.tensor.matmul(
                            ps[32 * j:32 * (j + 1), :],
                            lhsT=xt[kb][:, fl, :],
                            rhs=wt[kb][:, fp_ := fl - f0, :],
                            start=(kb == 0), stop=(kb == 1),
                            tile_position=(0, 32 * j))
                # drain every group col-range whose mms are now complete
                # (PSUM -> SBUF on ACT; keeps DVE free for W-scaling)
                for g in sorted(ps_of):
                    j_done = min(f1 - g * CG, CG)
                    j0 = drained_to[g]
                    if j_done > j0:
                        nc.scalar.copy(
                            out=out_sb[32 * j0:32 * j_done, g, :],
                            in_=ps_of[g][32 * j0:32 * j_done, :])
                        drained_to[g] = j_done
                    if j_done == CG:
                        del ps_of[g]
            # out pieces whose groups are fully drained go to DRAM on the
            # ACT HWDGE ring; small final pieces shrink the tail
            while next_out is not None and next_out[1] * CG <= f1:
                g0, g1 = next_out
                nc.scalar.dma_start(out=out_ap[:, g0:g1, :],
                                    in_=out_sb[:, g0:g1, :])
                next_out = next(out_iter, None)


def build_program(n_repeat=1, loop_k=None):
    nc = bacc.Bacc("TRN2", target_bir_lowering=False, debug=False,
                   num_devices=N_CORES)
    w = nc.dram_tensor("w", [N, FC, N], F32, kind="ExternalInput").ap()
    xs = nc.dram_tensor("xs", [N, FC, B], F32, kind="ExternalInput").ap()
    phm = nc.dram_tensor("phm", [2, N, N], F16 if K_PHM16 else F32,
                         kind="ExternalInput").ap()
    out = nc.dram_tensor("out", [CG, B, NG, N], F16 if K_OUT16 else F32,
                         kind="ExternalOutput").ap()

    with tile.TileContext(nc) as tc:
        # Warmup Sin activation outside the loop so the one-time ACT
        # table load (~1.3us) is not paid inside every iteration.  The
        # constant Sin bias (-pi/2) is also hoisted so its memset does not
        # occupy the Pool queue (which emits the SWDGE W stream) per
        # iteration.
        with tc.tile_pool(name="constp", bufs=1) as constp:
            bias_t = constp.tile([128, 1], F32)
            nc.vector.memset(bias_t, -HALF_PI)
            warm_t = constp.tile([128, 1], F32)
            nc.scalar.activation(out=warm_t, in_=bias_t,
                                 func=mybir.ActivationFunctionType.Sin)
            if loop_k is not None:
                with tc.For_i(0, loop_k, 1):
                    for _ in range(n_repeat):
                        build_body(tc, w, xs, phm, out, bias_t)
            else:
                for _ in range(n_repeat):
                    build_body(tc, w, xs, phm, out, bias_t)
    nc.compile()
    return nc


def host_prep(phase, src, dst):
    """Per-(s,d) multiplicity / out-degree normalization from the integer
    edge tensors.  Returns ms [N, N] float32 with ms[s,d] = M[s,d]/norm[d]."""
    src = np.asarray(src).astype(np.int64)
    dst = np.asarray(dst).astype(np.int64)
    counts = np.bincount(src, minlength=N).astype(np.float64)
    norm = np.maximum(counts, 1.0)                      # per-node out-degree
    mult = np.bincount(src * N + dst, minlength=N * N).astype(np.float64)
    mult = mult.reshape(N, N)
    ms = (mult / norm[None, :]).astype(np.float32)
    return ms


_PROGRAM_CACHE = {}


def get_program(n_repeat=1, loop_k=None):
    key = (n_repeat, loop_k)
    if key not in _PROGRAM_CACHE:
        _PROGRAM_CACHE[key] = build_program(n_repeat, loop_k)
    return _PROGRAM_CACHE[key]


def make_in_maps(node_features, W, phase, src, dst):
    node_features = np.asarray(node_features, dtype=np.float32)
    W = np.asarray(W, dtype=np.float32)
    phase = np.asarray(phase, dtype=np.float32)
    ms = host_prep(phase, src, dst)
    phm = np.ascontiguousarray(np.stack([phase, ms], axis=0))
    if K_PHM16:
        phm = phm.astype(np.float16)
    in_maps = []
    for c in range(N_CORES):
        fsl = slice(c * FC, (c + 1) * FC)
        in_maps.append({
            # [s, d, f] -> [s, f, d]
            "w": np.ascontiguousarray(W[:, :, fsl].transpose(0, 2, 1)),
            # [b, s, f] -> [s, f, b]
            "xs": np.ascontiguousarray(
                node_features[:, :, fsl].transpose(1, 2, 0)),
            "phm": phm,
        })
    return in_maps


def unshard(res_out):
    """Per-core out [CG, B, NG, N] (j, b, g, d) -> [B, N, FC] with f=4g+j."""
    return np.ascontiguousarray(
        res_out.astype(np.float32).transpose(1, 3, 2, 0).reshape(B, N, FC))


def kernel(node_features, W, phase, src, dst):
    nc = get_program(1)
    in_maps = make_in_maps(node_features, W, phase, src, dst)
    res = run_bass_kernel_spmd(nc, in_maps, list(range(N_CORES)))
    return np.concatenate(
        [unshard(res.results[c]["out"]) for c in range(N_CORES)], axis=2)


# revision 28
# speedup vs baseline: 2.2317x; 1.0112x over previous
"""Trainium2 Bass kernel for nn_EntanglementPropagator (gnn_message_passing).

Math: with C[s,d] = cos(phase[s,d]) * M[s,d] / norm[d]  (M = edge
multiplicity, norm = clamped out-degree), the reference reduces to

    out[b,d,f] = sum_s (W[s,d,f] * C[s,d]) * x[b,s,f]

i.e. F independent [B,N] x [N,N] matmuls (contraction over source node s).

Sharding: FEATURE-dim split across the 8 cores (core c owns f in
[32c, 32c+32)).  Unlike dst-sharding, every input byte is read exactly
once across the machine: per core W slice 8 MB + x slice 1 MB + phase/ms
0.5 MB + out 1 MB ~= 10.5 MB, which at ~358 GB/s per-core HBM bandwidth
sets a ~29 us roofline (vs 17.5 MB -> ~49 us for dst-sharding).

Per-core compute structure (per feature f): out[b,:] += x[s,b]^T @ Wc[s,:]
with s split in two 128-partition blocks accumulated in PSUM.
Key design points:
  * W and x are cast fp32 -> bf16 DURING the DMA (SWDGE/gpsimd cast-DMA),
    so the DVE only does the C-scale multiply at bf16 2x rate (~10 us)
    instead of fp32 1x (~17.5 us), and SBUF traffic halves.
  * b (=32) sits on PSUM partitions; 4 features are packed into the four
    32-wide PE column groups via tile_position=(0,32j), so PSUM drains see
    all 128 partitions ([128,256] ACT copies, ~2.7 us total) instead of
    32-partition tiles (4x slower).
  * bf16 matmul streams rhs=Wc at 1 cycle/column (vs 4 for fp32): PE ~2-3 us.
  * The two s-halves accumulate in PSUM (start/stop pair per feature),
    which requires the (kb0,kb1) pieces of a f-chunk to arrive adjacently
    on the gpsimd DMA queue.
  * Tail pieces of the W stream are smaller so little work remains after
    the last input byte lands; out drains to DRAM in two halves on the
    ACT HWDGE ring.

The host only does layout work (slice/transpose/stack) plus preprocessing
of the *integer* edge tensors (multiplicity/degree bincounts); cos() and
all heavy FP math run on device.
"""

import numpy as np

import concourse.mybir as mybir
import concourse.tile as tile
from concourse import bacc
from concourse.bass_utils import run_bass_kernel_spmd

N = 256          # nodes
F = 256          # feature dim
B = 32           # batch
N_CORES = 8
FC = F // N_CORES        # features per core = 32
KB = 2                   # source-node partition blocks (s: 2 x 128)
CG = 4                   # features packed per PSUM tile (PE col groups)
NG = FC // CG            # feature groups per core = 8
F32 = mybir.dt.float32
BF16 = mybir.dt.bfloat16

import os
K_DMA_ONLY = os.environ.get("K_DMA_ONLY", "0") == "1"   # skip compute (A/B)
K_WMODE = os.environ.get("K_WMODE", "cast")             # cast | plain
K_EMPTY = os.environ.get("K_EMPTY", "0") == "1"         # empty loop body
K_BIG = os.environ.get("K_BIG", "0") == "1"             # one 4MB W piece/kb
K_PHM16 = os.environ.get("K_PHM16", "1") == "1"         # phase/ms as fp16
K_OUT16 = os.environ.get("K_OUT16", "1") == "1"         # out as fp16 (host widens)
F16 = mybir.dt.float16

HALF_PI = float(np.pi / 2.0)

# W stream f-chunks (per kb).  16+8+4+2+2 = 32; big head pieces amortize
# per-DMA overheads, small tail pieces leave little work after the last
# input byte lands.
FCHUNKS = [(0, 16), (16, 24), (24, 28), (28, 30), (30, 32)]
FMAX = 16                # largest chunk width (wpool tile size)
# out-DMA pieces (group ranges), issued as soon as their groups are drained;
# the final pieces are single 128KB groups to shrink the tail.
OUT_PIECES = [(0, 4), (4, 6), (6, 7), (7, 8)]


def build_body(tc, w, xs, phm, out, bias_t):
    """Emit one iteration of the kernel body.

    w   [N, FC, N]   DRAM fp32 - W[:, :, fsl] transposed to [s, f, d]
    xs  [N, FC, B]   DRAM fp32 - node_features[:, :, fsl] as [s, f, b]
    phm [2, N, N]    DRAM fp32 - phase and M/norm scale (int-derived)
    out [CG, B, NG, N] DRAM fp32 - psum-partition-major output layout:
                     out[j, b, g, d] = result[b, d, f=4g+j]
    """
    nc = tc.nc

    if K_EMPTY:
        with tc.tile_pool(name="epool", bufs=2) as epool:
            et = epool.tile([128, 1], F32)
            nc.vector.memset(et, 0.0)
        return

    with (
        tc.tile_pool(name="cpool", bufs=2) as cpool,
        tc.tile_pool(name="xpool", bufs=2) as xpool,
        tc.tile_pool(name="wpool", bufs=2 if K_BIG else 4) as wpool,
        tc.tile_pool(name="opool", bufs=1) as opool,
        tc.tile_pool(name="ppool", bufs=4, space="PSUM") as ppool,
    ):
        # --- per-(s,d) scale C = cos(phase) * M/norm, bf16, layout
        # [s_part, d] per s-half.  The Sin LUT is only accurate on
        # ~[-pi, pi], so use cos(x) = 2*sin^2(x/2 - pi/2) - 1.
        phdt = F16 if K_PHM16 else F32
        phm_t = cpool.tile([128, 2, KB, N], phdt, tag="phm")
        nc.sync.dma_start(
            out=phm_t, in_=phm.rearrange("t (k p) d -> p t k d", k=KB))
        c_t = {}
        for kb in range(KB):
            c = cpool.tile([128, N], F32, tag="c")
            nc.scalar.activation(out=c, in_=phm_t[:, 0, kb, :],
                                 func=mybir.ActivationFunctionType.Sin,
                                 bias=bias_t, scale=0.5)
            nc.vector.tensor_mul(out=c, in0=c, in1=c)
            nc.vector.tensor_scalar(out=c, in0=c, scalar1=2.0, scalar2=-1.0,
                                    op0=mybir.AluOpType.mult,
                                    op1=mybir.AluOpType.add)
            c16 = cpool.tile([128, N], BF16, tag="c16")
            nc.vector.tensor_mul(out=c16, in0=c, in1=phm_t[:, 1, kb, :])
            c_t[kb] = c16

        # --- x: fp32 -> bf16 cast-DMA, both s-halves in one transfer.
        x_t = xpool.tile([128, KB, FC, B], BF16, tag="x")
        nc.gpsimd.dma_start(
            out=x_t, in_=xs.rearrange("(k p) f b -> p k f b", k=KB))
        xt = {kb: x_t[:, kb] for kb in range(KB)}

        # out_sb [p=(j,b), g, d]: drains land partition-major; the host
        # unshards (transpose) so the out DMA is fully contiguous.
        out_sb = opool.tile([128, NG, N], F16 if K_OUT16 else F32)
        out_ap = out.rearrange("j b g d -> (j b) g d")
        if K_DMA_ONLY:
            nc.vector.memset(out_sb, 0.0)

        # --- stream W pieces and compute.  A group g (4 features) owns one
        # PSUM tile; its j-th col-block completes as soon as the piece
        # holding feature 4g+j has been scaled.  Groups drain (possibly in
        # partition-halves, for groups split across pieces) as their mms
        # finish, and out pieces go to DRAM as their groups drain.
        ps_of = {}                 # g -> psum tile
        drained_to = {}            # g -> next j to drain
        out_iter = iter(OUT_PIECES)
        next_out = next(out_iter, None)
        w_r = w.rearrange("(k p) f d -> p k f d", k=KB)
        for f0, f1 in ([(0, FC)] if K_BIG else FCHUNKS):
            fw = f1 - f0
            wdt = BF16 if K_WMODE == "cast" else F32
            t = wpool.tile([128, KB, FC if K_BIG else FMAX, N], wdt, tag="w")
            t = t[:, :, :fw, :]
            if K_WMODE == "cast":
                nc.gpsimd.dma_start(out=t, in_=w_r[:, :, f0:f1, :])
            else:
                nc.sync.dma_start(out=t, in_=w_r[:, :, f0:f1, :])
            wt = {}
            for kb in range(KB):
                wt[kb] = t[:, kb]
                if not K_DMA_ONLY:
                    # Wc = W * C (broadcast C over f) on DVE (bf16 2x rate)
                    nc.vector.tensor_mul(
                        out=wt[kb], in0=wt[kb],
                        in1=c_t[kb][:, None, :].broadcast_to([128, fw, N]))

            if not K_DMA_ONLY:
                for fl in range(f0, f1):
                    g, j = divmod(fl, CG)
                    if g not in ps_of:
                        ps_of[g] = ppool.tile([128, N], F32, name="ps",
                                              tag="ps")
                        drained_to[g] = 0
                    ps = ps_of[g]
                    for kb in range(KB):
                        nc.tensor.matmul(
                            ps[32 * j:32 * (j + 1), :],
                            lhsT=xt[kb][:, fl, :],
                            rhs=wt[kb][:, fp_ := fl - f0, :],
                            start=(kb == 0), stop=(kb == 1),
                            tile_position=(0, 32 * j))
                # drain every group col-range whose mms are now complete
                # (PSUM -> SBUF on ACT; keeps DVE free for W-scaling)
                for g in sorted(ps_of):
                    j_done = min(f1 - g * CG, CG)
                    j0 = drained_to[g]
                    if j_done > j0:
                        nc.scalar.copy(
                            out=out_sb[32 * j0:32 * j_done, g, :],
                            in_=ps_of[g][32 * j0:32 * j_done, :])
                        drained_to[g] = j_done
                    if j_done == CG:
                        del ps_of[g]
            # out pieces whose groups are fully drained go to DRAM on the
            # ACT HWDGE ring; small final pieces shrink the tail
            while next_out is not None and next_out[1] * CG <= f1:
                g0, g1 = next_out
                nc.scalar.dma_start(out=out_ap[:, g0:g1, :],
                                    in_=out_sb[:, g0:g1, :])
                next_out = next(out_iter, None)


def build_program(n_repeat=1, loop_k=None):
    nc = bacc.Bacc("TRN2", target_bir_lowering=False, debug=False,
                   num_devices=N_CORES)
    w = nc.dram_tensor("w", [N, FC, N], F32, kind="ExternalInput").ap()
    xs = nc.dram_tensor("xs", [N, FC, B], F32, kind="ExternalInput").ap()
    phm = nc.dram_tensor("phm", [2, N, N], F16 if K_PHM16 else F32,
                         kind="ExternalInput").ap()
    out = nc.dram_tensor("out", [CG, B, NG, N], F16 if K_OUT16 else F32,
                         kind="ExternalOutput").ap()

    with tile.TileContext(nc) as tc:
        # Warmup Sin activation outside the loop so the one-time ACT
        # table load (~1.3us) is not paid inside every iteration.  The
        # constant Sin bias (-pi/2) is also hoisted so its memset does not
        # occupy the Pool queue (which emits the SWDGE W stream) per
        # iteration.
        with tc.tile_pool(name="constp", bufs=1) as constp:
            bias_t = constp.tile([128, 1], F32)
            nc.vector.memset(bias_t, -HALF_PI)
            warm_t = constp.tile([128, 1], F32)
            nc.scalar.activation(out=warm_t, in_=bias_t,
                                 func=mybir.ActivationFunctionType.Sin)
            if loop_k is not None:
                with tc.For_i(0, loop_k, 1):
                    for _ in range(n_repeat):
                        build_body(tc, w, xs, phm, out, bias_t)
            else:
                for _ in range(n_repeat):
                    build_body(tc, w, xs, phm, out, bias_t)
    nc.compile()
    return nc


def host_prep(phase, src, dst):
    """Per-(s,d) multiplicity / out-degree normalization from the integer
    edge tensors.  Returns ms [N, N] float32 with ms[s,d] = M[s,d]/norm[d]."""
    src = np.asarray(src).astype(np.int64)
    dst = np.asarray(dst).astype(np.int64)
    counts = np.bincount(src, minlength=N).astype(np.float64)
    norm = np.maximum(counts, 1.0)                      # per-node out-degree
    mult = np.bincount(src * N + dst, minlength=N * N).astype(np.float64)
    mult = mult.reshape(N, N)
    ms = (mult / norm[None, :]).astype(np.float32)
    return ms


_PROGRAM_CACHE = {}


def get_program(n_repeat=1, loop_k=None):
    key = (n_repeat, loop_k)
    if key not in _PROGRAM_CACHE:
        _PROGRAM_CACHE[key] = build_program(n_repeat, loop_k)
    return _PROGRAM_CACHE[key]


def make_in_maps(node_features, W, phase, src, dst):
    node_features = np.asarray(node_features, dtype=np.float32)
    W = np.asarray(W, dtype=np.float32)
    phase = np.asarray(phase, dtype=np.float32)
    ms = host_prep(phase, src, dst)
    phm = np.ascontiguousarray(np.stack([phase, ms], axis=0))
    if K_PHM16:
        phm = phm.astype(np.float16)
    in_maps = []
    for c in range(N_CORES):
        fsl = slice(c * FC, (c + 1) * FC)
        in_maps.append({
            # [s, d, f] -> [s, f, d]
            "w": np.ascontiguousarray(W[:, :, fsl].transpose(0, 2, 1)),
            # [b, s, f] -> [s, f, b]
            "xs": np.ascontiguousarray(
                node_features[:, :, fsl].transpose(1, 2, 0)),
            "phm": phm,
        })
    return in_maps


def unshard(res_out):
    """Per-core out [CG, B, NG, N] (j, b, g, d) -> [B, N, FC] with f=4g+j."""
    return np.ascontiguousarray(
        res_out.astype(np.float32).transpose(1, 3, 2, 0).reshape(B, N, FC))


def kernel(node_features, W, phase, src, dst):
    nc = get_program(1)
    in_maps = make_in_maps(node_features, W, phase, src, dst)
    res = run_bass_kernel_spmd(nc, in_maps, list(range(N_CORES)))
    return np.concatenate(
        [unshard(res.results[c]["out"]) for c in range(N_CORES)], axis=2)
